# revision 30
# baseline (speedup 1.0000x reference)
"""Trainium2 kernel for CompactBilinearLayer (count-sketch bilinear pooling).

Math: reference computes y = l2norm(signed_sqrt(sum_hw Re IFFT(FFT(x@M1)*FFT(x@M2)))).
Since M1/M2 are count-sketch matrices (one +-1 per row), FFT(x@M1) == x @ A1 with
A1[c,k] = s1[c] * exp(-2pi i h1[c] k / P) — a dense [512, K] matrix computable on the
host from M1 in O(C*K). The IFFT is linear, so the spatial sum moves before it.
Hermitian symmetry means only k = 0..4096 are needed.  Per core (4 batch elements,
784 spatial positions — fully batch-local, no collectives):
  A: P1/P2 projections = A^T @ x^T, single-pass bf16 matmuls; per-component
     pairs of 1-bank [128,392] PSUM tiles (7-deep ring) so Act evacuation
     copies start right after each half-group and never gate the next tile
  B: S[k,b] = sum_t (P1*P2) per batch via fused DVE scalar_tensor_tensor
     (product+reduce in one op), one PSUM + one SBUF operand each
  C: IFFT via two-step factorization n=64q+s: GpSimd computes the twiddle
     products and folds them to U = u1-u2, V = v1+v2 (f32r), PE accumulates
     just 2 f32r matmuls over k%128 into psy.  Stage-C matmuls are emitted
     3 iterations late so the in-order PE queue never waits on the
     DVE->GpSimd chain.
  D: signed sqrt + per-batch L2 norm + store
The k=4096 bin tile runs FIRST (it only needs x, not the streamed A tiles),
overlapping the startup DMA window.  A-tile DMAs alternate between the two
hardware DGE queues (Sync + Act) since one queue caps out at ~87 GB/s.
"""
import numpy as np

P = 8192
C = 512
FT = 33            # frequency tiles of 128 -> 4224 slots >= 4097
NSLOT = FT * 128
NCORES = 8
BPC = 4            # batch elems per core
HW = 196           # spatial positions per batch elem
T = BPC * HW       # 784 positions per core
HT = T // 2        # 392, one PSUM bank of f32
B = 32

_CACHE = {}


def _build_program():
    import concourse.bass as bass
    import concourse.tile as tile
    from concourse import bacc, mybir

    f32 = mybir.dt.float32
    f32r = mybir.dt.float32r
    bf16 = mybir.dt.bfloat16
    nc = bacc.Bacc("TRN2", target_bir_lowering=False, debug=False,
                   num_devices=NCORES)

    a_d = nc.dram_tensor("a", [FT - 1, 128, 4, 512], bf16, kind="ExternalInput").ap()
    a46_d = nc.dram_tensor("a46", [128, 4, 2], bf16, kind="ExternalInput").ap()
    x_d = nc.dram_tensor("x", [128, 4, T], bf16, kind="ExternalInput").ap()
    cphi_d = nc.dram_tensor("cphi", [128, FT, 64], f32, kind="ExternalInput").ap()
    sphi_d = nc.dram_tensor("sphi", [128, FT, 64], f32, kind="ExternalInput").ap()
    cosa_d = nc.dram_tensor("cosa", [128, 128], bf16, kind="ExternalInput").ap()
    nsina_d = nc.dram_tensor("nsina", [128, 128], bf16, kind="ExternalInput").ap()
    y_d = nc.dram_tensor("y", [BPC, P], f32, kind="ExternalOutput").ap()

    mult = mybir.AluOpType.mult
    subtract = mybir.AluOpType.subtract
    add = mybir.AluOpType.add
    bypass = mybir.AluOpType.bypass
    Act = mybir.ActivationFunctionType

    with tile.TileContext(nc) as tc:
        with (
            tc.tile_pool(name="const", bufs=1) as const,
            tc.tile_pool(name="apool", bufs=13) as apool,
            tc.tile_pool(name="pst", bufs=7, space="PSUM") as pstpool,
            tc.tile_pool(name="psyp", bufs=1, space="PSUM") as psypool,
            tc.tile_pool(name="scr", bufs=4) as scr,
            tc.tile_pool(name="uv", bufs=7) as uvpool,
        ):
            a46_sb = const.tile([128, 4, 2], bf16)
            nc.sync.dma_start(a46_sb[:], a46_d)
            x_sb = const.tile([128, 4, T], bf16)
            # two descriptors: the first ft46 matmuls only touch t < HT,
            # so they can start ~1us before the second half lands
            nc.sync.dma_start(x_sb[:, :, 0:HT], x_d[:, :, 0:HT])
            nc.sync.dma_start(x_sb[:, :, HT:T], x_d[:, :, HT:T])
            cphi_sb = const.tile([128, FT, 64], f32)
            sphi_sb = const.tile([128, FT, 64], f32)
            cosa_sb = const.tile([128, 128], bf16)
            nsina_sb = const.tile([128, 128], bf16)
            ones_sb = const.tile([128, 1], bf16)
            nc.vector.memset(ones_sb[:], 1.0)
            # preload the Abs/Sqrt/Sign activation tables during the initial
            # DMA window so stage D doesn't pay the ~2.6us table switch
            warm = const.tile([1, 1], f32)
            nc.vector.memset(warm[:], 1.0)
            wo = const.tile([1, 1], f32)
            nc.scalar.activation(wo[:], warm[:], Act.Abs)
            nc.scalar.activation(wo[:], wo[:], Act.Sqrt)
            nc.scalar.activation(wo[:], wo[:], Act.Sign)
            sre_sb = const.tile([128, FT * 4], f32)
            sim_sb = const.tile([128, FT * 4], f32)
            nc.vector.memset(sre_sb[:, (FT - 1) * 4:], 0.0)
            nc.vector.memset(sim_sb[:, (FT - 1) * 4:], 0.0)
            sA_sb = const.tile([128, FT * 4], f32)
            sB_sb = const.tile([128, FT * 4], f32)
            sC_sb = const.tile([128, FT * 4], f32)
            sD_sb = const.tile([128, FT * 4], f32)

            psy = psypool.tile([128, BPC * 64], f32, tag="psy")

            n_emitted = [0]
            N_CMM = 2 * FT  # total stage-C matmuls

            def emit_stage_c(ft, us):
                U, V = us
                nc.tensor.matmul(psy[:], cosa_sb[:],
                                 U[:].rearrange("p b s -> p (b s)"),
                                 start=(n_emitted[0] == 0), stop=False)
                nc.tensor.matmul(psy[:], nsina_sb[:],
                                 V[:].rearrange("p b s -> p (b s)"),
                                 start=False,
                                 stop=(n_emitted[0] == N_CMM - 2))
                n_emitted[0] += 2

            pend = []

            def _bcast(src, ft):
                return src[:, ft, :][:, None, :].broadcast_to([128, BPC, 64])

            def _sbcast(src, fsl):
                return src[:, fsl][:, :, None].broadcast_to([128, BPC, 64])

            # twiddle products on GpSimd (u = phi * S, broadcast both ways),
            # folded to U = u1-u2, V = v1+v2 so the PE only needs 2 IFFT
            # matmuls per tile.  Split in two halves: the re-products depend
            # only on S_re and are emitted mid-way through the DVE chain so
            # GpSimd overlaps the remaining stts (shortens the serial tail).
            def emit_uv_re(ft):
                fsl = slice(ft * 4, (ft + 1) * 4)
                u1 = uvpool.tile([128, BPC, 64], f32, tag="u1", name=f"u1_{ft}")
                v1 = uvpool.tile([128, BPC, 64], f32, tag="v1", name=f"v1_{ft}")
                nc.gpsimd.tensor_tensor(u1[:], _bcast(cphi_sb, ft),
                                        _sbcast(sre_sb, fsl), op=mult)
                nc.gpsimd.tensor_tensor(v1[:], _bcast(sphi_sb, ft),
                                        _sbcast(sre_sb, fsl), op=mult)
                return u1, v1

            def emit_uv_im(ft, u1, v1):
                # for the final tile the GpSimd chain is the serial tail;
                # the DVE is idle by then and ~1.7x faster per op
                eng = nc.vector if ft == FT - 2 else nc.gpsimd
                fsl = slice(ft * 4, (ft + 1) * 4)
                u2 = uvpool.tile([128, BPC, 64], f32, tag="u2", name=f"u2_{ft}")
                v2 = uvpool.tile([128, BPC, 64], f32, tag="v2", name=f"v2_{ft}")
                Ut = uvpool.tile([128, BPC, 64], bf16, tag="U", name=f"U_{ft}")
                Vt = uvpool.tile([128, BPC, 64], bf16, tag="V", name=f"V_{ft}")
                eng.tensor_tensor(u2[:], _bcast(sphi_sb, ft),
                                  _sbcast(sim_sb, fsl), op=mult)
                eng.tensor_tensor(v2[:], _bcast(cphi_sb, ft),
                                  _sbcast(sim_sb, fsl), op=mult)
                eng.tensor_tensor(Ut[:], u1[:], u2[:], op=subtract)
                eng.tensor_tensor(Vt[:], v1[:], v2[:], op=add)
                pend.append((ft, (Ut, Vt)))

            def emit_uv46():
                # k=4096 bin: S_im == 0 there, so U = cphi*S_re, V = sphi*S_re
                ft = FT - 1
                fsl = slice(ft * 4, (ft + 1) * 4)
                Ut = uvpool.tile([128, BPC, 64], bf16, tag="U", name="U_46")
                Vt = uvpool.tile([128, BPC, 64], bf16, tag="V", name="V_46")
                nc.gpsimd.tensor_tensor(Ut[:], _bcast(cphi_sb, ft),
                                        _sbcast(sre_sb, fsl), op=mult)
                nc.gpsimd.tensor_tensor(Vt[:], _bcast(sphi_sb, ft),
                                        _sbcast(sre_sb, fsl), op=mult)
                pend.append((ft, (Ut, Vt)))

            # k=4096 bin tile (ft == FT-1) runs first: it needs only x.
            for ft in [FT - 1] + list(range(FT - 1)):
                if ft == FT - 1:
                    # k=4096 bin: P1/P2 via 1-column matmuls onto partition 0.
                    # cphi row 4096 is (-1)^s/P (sphi row 0, cosa row 0 all-1,
                    # nsina row 0 all-0), so the generic twiddle+IFFT path
                    # below handles this bin exactly.
                    ps46 = {}
                    for hi, c0 in ((0, 0), (1, HT)):
                        for col in (0, 1):
                            ps = pstpool.tile([128, HT], f32, tag="pst",
                                              name=f"ps46_{col}_{hi}")
                            for ck in range(4):
                                nc.tensor.matmul(
                                    ps[0:1, :],
                                    a46_sb[:, ck, col:col + 1],
                                    x_sb[:, ck, c0:c0 + HT],
                                    start=(ck == 0),
                                    stop=(ck == 3),
                                )
                            ps46[(col, hi)] = ps
                    c46 = {}
                    for col in (0, 1):
                        for hi in (0, 1):
                            c_t = scr.tile([128, HT], bf16,
                                           tag=f"c{2 + col}{hi}",
                                           name=f"c46_{col}_{hi}")
                            nc.scalar.activation(c_t[0:1, :],
                                                 ps46[(col, hi)][0:1, :],
                                                 Act.Copy)
                            c46[(col, hi)] = c_t
                    for bl in range(BPC):
                        idx = ft * 4 + bl
                        hi, s0 = bl // 2, (bl % 2) * HW
                        seg = slice(s0, s0 + HW)
                        sc = scr.tile([128, HW], bf16, tag="sc0",
                                      name=f"sc46_{bl}")
                        nc.vector.scalar_tensor_tensor(
                            sc[0:1, :], c46[(0, hi)][0:1, seg], 1.0,
                            c46[(1, hi)][0:1, seg],
                            bypass, mult,
                            accum_out=sre_sb[0:1, idx:idx + 1])
                else:
                    a_t = apool.tile([128, 4, 512], bf16, tag="a")
                    nc.sync.dma_start(a_t[:], a_d[ft])
                if ft == 4:
                    # bulky twiddle constants: sync-queue position after a4
                    # so x/a0..a4 get the full HBM bandwidth first (GpSimd
                    # only needs them ~4 periods in).  The scheduler keeps
                    # same-queue DMA emission order; the Act queue does not.
                    # All uv emission for earlier tiles is deferred to here:
                    # a cphi/sphi read emitted before these dma_starts would
                    # order the DMA after the read and compute on garbage.
                    nc.sync.dma_start(cphi_sb[:], cphi_d)
                    nc.sync.dma_start(sphi_sb[:], sphi_d)
                    nc.sync.dma_start(cosa_sb[:], cosa_d)
                    nc.sync.dma_start(nsina_sb[:], nsina_d)
                    emit_uv46()
                    for f in range(4):
                        emit_uv_im(f, *emit_uv_re(f))
                cpy = {}
                # Act evacuates every 1-bank half to SBUF bf16 right after
                # its 4 matmuls: all-SBUF packed bf16 stt operands unlock
                # the DVE 2x perf mode (PSUM or fp32 operands forfeit it)
                for m in (2, 3, 0, 1) if ft < FT - 1 else ():
                    msl = slice(m * 128, (m + 1) * 128)
                    for hi, c0 in ((0, 0), (1, HT)):
                        ps = pstpool.tile([128, HT], f32, tag="pst",
                                          name=f"ps{m}_{hi}_{ft}")
                        for ck in range(4):
                            nc.tensor.matmul(
                                ps[:, :],
                                a_t[:, ck, msl],
                                x_sb[:, ck, c0:c0 + HT],
                                start=(ck == 0),
                                stop=(ck == 3),
                            )
                        c_m = scr.tile([128, HT], bf16, tag=f"c{m}{hi}",
                                       name=f"c{m}{hi}_{ft}")
                        nc.scalar.activation(c_m[:], ps[:], Act.Copy)
                        cpy[(m, hi)] = c_m

                # A=sum p0*p2, B=sum p1*p3, C=sum p0*p3, D=sum p1*p2.
                # A,B first so S_re = A-B is ready mid-chain and GpSimd's
                # re-products overlap the C,D stts.
                def stt_chain(tg, pa, cb, dst):
                    for bl in range(BPC):
                        idx = ft * 4 + bl
                        hi, s0 = bl // 2, (bl % 2) * HW
                        seg = slice(s0, s0 + HW)
                        sc = scr.tile([128, HW], bf16, tag=tg,
                                      name=f"{tg}_{ft}_{bl}")
                        nc.vector.scalar_tensor_tensor(
                            sc[:], cpy[(pa, hi)][:, seg], 1.0,
                            cpy[(cb, hi)][:, seg],
                            bypass, mult,
                            accum_out=dst[:, idx:idx + 1])

                fsl = slice(ft * 4, (ft + 1) * 4)
                if ft < FT - 1:
                    stt_chain("sc0", 0, 2, sA_sb)
                    stt_chain("sc1", 1, 3, sB_sb)
                    nc.vector.tensor_sub(sre_sb[:, fsl], sA_sb[:, fsl],
                                         sB_sb[:, fsl])
                    u1 = v1 = None
                    if ft >= 4:
                        u1, v1 = emit_uv_re(ft)
                    stt_chain("sc2", 0, 3, sC_sb)
                    stt_chain("sc3", 1, 2, sD_sb)
                    nc.vector.tensor_add(sim_sb[:, fsl], sC_sb[:, fsl],
                                         sD_sb[:, fsl])
                    if ft >= 4:
                        emit_uv_im(ft, u1, v1)

                # emit IFFT matmuls 3 iterations late to keep the PE queue fed
                while len(pend) > 3:
                    emit_stage_c(*pend.pop(0))
            while pend:
                emit_stage_c(*pend.pop(0))

            # ---- stage D: signed sqrt, per-batch l2 norm, store ----
            # bf16 |Y| feeds the norm matmul (1-pass bf16 instead of the
            # double-pass fp32 LOW_HIGH) and the Sqrt; error ~0.2% final
            absy = const.tile([128, BPC * 64], bf16)
            nc.scalar.activation(absy[:], psy[:], Act.Abs)
            sqy = const.tile([128, BPC * 64], f32)
            nc.scalar.activation(sqy[:], absy[:], Act.Sqrt)
            sgn = const.tile([128, BPC * 64], f32)
            nc.scalar.activation(sgn[:], psy[:], Act.Sign)
            ys = const.tile([128, BPC * 64], f32)
            nc.vector.tensor_mul(ys[:], sqy[:], sgn[:])

            # norm^2 per batch = sum_p y^2 = sum_p |Y|  (Y = pre-sqrt value)
            psn = pstpool.tile([128, BPC * 64], f32, tag="pst", name="psn")
            nc.tensor.matmul(psn[0:1, :], ones_sb[:], absy[:],
                             start=True, stop=True)
            nsq = const.tile([1, BPC], f32)
            nc.vector.reduce_sum(
                out=nsq[:],
                in_=psn[0:1, :].rearrange("p (b s) -> p b s", b=BPC),
                axis=mybir.AxisListType.X,
            )
            nc.vector.tensor_scalar_max(nsq[:], nsq[:], 1e-10)
            sqn = const.tile([1, BPC], f32)
            nc.scalar.activation(sqn[:], nsq[:], Act.Sqrt)
            invn = const.tile([1, BPC], f32)
            nc.vector.reciprocal(invn[:], sqn[:])
            invn16 = const.tile([1, BPC], bf16)
            nc.scalar.activation(invn16[:], invn[:], Act.Copy)

            onesrow = const.tile([1, 128], bf16)
            nc.vector.memset(onesrow[:], 1.0)
            psb = pstpool.tile([128, BPC], f32, tag="pst", name="psb")
            nc.tensor.matmul(psb[:, 0:BPC], onesrow[0:1, :], invn16[0:1, :],
                             start=True, stop=True)
            inv_b = psb[:, 0:BPC][:, :, None].broadcast_to([128, BPC, 64])
            fin = const.tile([128, BPC * 64], f32)
            nc.vector.tensor_tensor(
                fin[:].rearrange("p (b s) -> p b s", b=BPC),
                ys[:].rearrange("p (b s) -> p b s", b=BPC),
                inv_b,
                op=mult,
            )
            # single descriptor for all 4 batch rows
            nc.sync.dma_start(
                y_d.rearrange("b (q s) -> q b s", q=128),
                fin[:].rearrange("p (b s) -> p b s", b=BPC),
            )

    nc.compile()
    return nc


def _to_bf16(a):
    import ml_dtypes
    return np.asarray(a, np.float32).astype(ml_dtypes.bfloat16)


def _host_prep(x, M1, M2):
    x = np.ascontiguousarray(np.asarray(x, np.float32))
    M1 = np.asarray(M1, np.float32)
    M2 = np.asarray(M2, np.float32)

    h1 = np.argmax(np.abs(M1), axis=1)
    s1 = M1[np.arange(C), h1].astype(np.float64)
    h2 = np.argmax(np.abs(M2), axis=1)
    s2 = M2[np.arange(C), h2].astype(np.float64)

    k = np.arange(NSLOT, dtype=np.float64)
    valid = k <= P // 2
    ang1 = 2 * np.pi * np.outer(h1.astype(np.float64), k) / P
    ang2 = 2 * np.pi * np.outer(h2.astype(np.float64), k) / P
    # a[ft, c, m*128 + j]: m in (A1re, A1im, A2re, A2im), freq = ft*128 + j
    a = np.empty((FT - 1, C, 512), np.float32)
    a1re = (s1[:, None] * np.cos(ang1) * valid).astype(np.float32)
    a1im = (-s1[:, None] * np.sin(ang1) * valid).astype(np.float32)
    a2re = (s2[:, None] * np.cos(ang2) * valid).astype(np.float32)
    a2im = (-s2[:, None] * np.sin(ang2) * valid).astype(np.float32)
    for ft in range(FT - 1):
        ksl = slice(ft * 128, (ft + 1) * 128)
        a[ft, :, 0:128] = a1re[:, ksl]
        a[ft, :, 128:256] = a1im[:, ksl]
        a[ft, :, 256:384] = a2re[:, ksl]
        a[ft, :, 384:512] = a2im[:, ksl]

    w = np.where(valid, 2.0 / P, 0.0)
    w[0] = 1.0 / P
    w[P // 2] = 1.0 / P
    s_idx = np.arange(64, dtype=np.float64)
    phi = 2 * np.pi * np.outer(k, s_idx) / P
    cphi = (w[:, None] * np.cos(phi)).astype(np.float32).reshape(FT, 128, 64)
    sphi = (w[:, None] * np.sin(phi)).astype(np.float32).reshape(FT, 128, 64)

    km = np.arange(128, dtype=np.float64)
    alpha = 2 * np.pi * np.outer(km, km) / 128
    cosa = np.cos(alpha).astype(np.float32)
    nsina = (-np.sin(alpha)).astype(np.float32)

    xt = np.ascontiguousarray(x.reshape(B * HW, C).T)  # [C, 6272]

    # pre-transposed, DMA-contiguous layouts
    a_t = np.ascontiguousarray(
        a.reshape(FT - 1, 4, 128, 512).transpose(0, 2, 1, 3))
    # k=4096 columns: a46[p, ck, 0/1] = s1/2[c] * cos(pi*h1/2[c]), c=ck*128+p
    a46 = np.stack([a1re[:, 4096], a2re[:, 4096]], axis=1)  # [C, 2]
    a46 = np.ascontiguousarray(a46.reshape(4, 128, 2).transpose(1, 0, 2))
    cphi_t = np.ascontiguousarray(cphi.transpose(1, 0, 2))  # [128p,FT,64]
    sphi_t = np.ascontiguousarray(sphi.transpose(1, 0, 2))
    xt_t = xt.reshape(4, 128, B * HW).transpose(1, 0, 2)    # [128p,4ck,T]
    return (_to_bf16(a_t), _to_bf16(a46), cphi_t, sphi_t, _to_bf16(cosa),
            _to_bf16(nsina), _to_bf16(xt_t))


def _make_in_maps(x, M1, M2):
    a, a46, cphi, sphi, cosa, nsina, xt = _host_prep(x, M1, M2)
    in_maps = []
    for r in range(NCORES):
        in_maps.append({
            "a": a,
            "a46": a46,
            "x": np.ascontiguousarray(xt[:, :, r * T:(r + 1) * T]),
            "cphi": cphi,
            "sphi": sphi,
            "cosa": cosa,
            "nsina": nsina,
        })
    return in_maps


def kernel(x, M1, M2):
    from concourse.bass_utils import run_bass_kernel_spmd

    if "nc" not in _CACHE:
        _CACHE["nc"] = _build_program()
    nc = _CACHE["nc"]

    in_maps = _make_in_maps(x, M1, M2)
    res = run_bass_kernel_spmd(nc, in_maps, core_ids=list(range(NCORES)))
    out = np.concatenate([res.results[r]["y"] for r in range(NCORES)], axis=0)
    return out.astype(np.float32)


# revision 33
# speedup vs baseline: 1.2971x; 1.2971x over previous
"""Trainium2 kernel for CompactBilinearLayer (count-sketch bilinear pooling).

Math: reference computes y = l2norm(signed_sqrt(sum_hw Re IFFT(FFT(x@M1)*FFT(x@M2)))).
Since M1/M2 are count-sketch matrices (one +-1 per row), FFT(x@M1) == x @ A1 with
A1[c,k] = s1[c] * exp(-2pi i h1[c] k / P) — a dense [512, K] matrix computable on the
host from M1 in O(C*K). The IFFT is linear, so the spatial sum moves before it.
Hermitian symmetry means only k = 0..4096 are needed.  Per core (4 batch elements,
784 spatial positions — fully batch-local, no collectives):
  A: P1/P2 projections = A^T @ x^T, single-pass bf16 matmuls; per-component
     pairs of 1-bank [128,392] PSUM tiles (7-deep ring) so Act evacuation
     copies start right after each half-group and never gate the next tile
  B: S[k,b] = sum_t (P1*P2) per batch via fused DVE scalar_tensor_tensor
     (product+reduce in one op), one PSUM + one SBUF operand each
  C: IFFT via two-step factorization n=64q+s: GpSimd computes the twiddle
     products and folds them to U = u1-u2, V = v1+v2 (f32r), PE accumulates
     just 2 f32r matmuls over k%128 into psy.  Stage-C matmuls are emitted
     3 iterations late so the in-order PE queue never waits on the
     DVE->GpSimd chain.
  D: signed sqrt + per-batch L2 norm + store
The k=4096 bin tile runs FIRST (it only needs x, not the streamed A tiles),
overlapping the startup DMA window.  A-tile DMAs alternate between the two
hardware DGE queues (Sync + Act) since one queue caps out at ~87 GB/s.
"""
import numpy as np

P = 8192
C = 512
FT = 33            # frequency tiles of 128 -> 4224 slots >= 4097
NSLOT = FT * 128
NCORES = 8
BPC = 4            # batch elems per core
HW = 196           # spatial positions per batch elem
T = BPC * HW       # 784 positions per core
HT = T // 2        # 392, one PSUM bank of f32
B = 32

_CACHE = {}


def _build_program():
    import concourse.bass as bass
    import concourse.tile as tile
    from concourse import bacc, mybir

    f32 = mybir.dt.float32
    f32r = mybir.dt.float32r
    bf16 = mybir.dt.bfloat16
    nc = bacc.Bacc("TRN2", target_bir_lowering=False, debug=False,
                   num_devices=NCORES)

    a_d = nc.dram_tensor("a", [FT - 1, 128, 4, 512], bf16, kind="ExternalInput").ap()
    a46_d = nc.dram_tensor("a46", [128, 4, 2], bf16, kind="ExternalInput").ap()
    x_d = nc.dram_tensor("x", [128, 4, T], bf16, kind="ExternalInput").ap()
    cphi_d = nc.dram_tensor("cphi", [128, FT, 64], f32, kind="ExternalInput").ap()
    sphi_d = nc.dram_tensor("sphi", [128, FT, 64], f32, kind="ExternalInput").ap()
    cosa_d = nc.dram_tensor("cosa", [128, 128], bf16, kind="ExternalInput").ap()
    nsina_d = nc.dram_tensor("nsina", [128, 128], bf16, kind="ExternalInput").ap()
    y_d = nc.dram_tensor("y", [BPC, P], f32, kind="ExternalOutput").ap()

    mult = mybir.AluOpType.mult
    subtract = mybir.AluOpType.subtract
    add = mybir.AluOpType.add
    bypass = mybir.AluOpType.bypass
    Act = mybir.ActivationFunctionType

    with tile.TileContext(nc) as tc:
        with (
            tc.tile_pool(name="const", bufs=1) as const,
            tc.tile_pool(name="apool", bufs=13) as apool,
            tc.tile_pool(name="pst", bufs=7, space="PSUM") as pstpool,
            tc.tile_pool(name="psyp", bufs=1, space="PSUM") as psypool,
            tc.tile_pool(name="scr", bufs=4) as scr,
            tc.tile_pool(name="uv", bufs=7) as uvpool,
        ):
            a46_sb = const.tile([128, 4, 2], bf16)
            nc.sync.dma_start(a46_sb[:], a46_d)
            x_sb = const.tile([128, 4, T], bf16)
            # two descriptors: the first ft46 matmuls only touch t < HT,
            # so they can start ~1us before the second half lands
            nc.sync.dma_start(x_sb[:, :, 0:HT], x_d[:, :, 0:HT])
            nc.sync.dma_start(x_sb[:, :, HT:T], x_d[:, :, HT:T])
            cphi_sb = const.tile([128, FT, 64], f32)
            sphi_sb = const.tile([128, FT, 64], f32)
            cosa_sb = const.tile([128, 128], bf16)
            nsina_sb = const.tile([128, 128], bf16)
            ones_sb = const.tile([128, 1], bf16)
            nc.vector.memset(ones_sb[:], 1.0)
            # preload the Abs/Sqrt/Sign activation tables during the initial
            # DMA window so stage D doesn't pay the ~2.6us table switch
            warm = const.tile([1, 1], f32)
            nc.vector.memset(warm[:], 1.0)
            wo = const.tile([1, 1], f32)
            nc.scalar.activation(wo[:], warm[:], Act.Abs)
            nc.scalar.activation(wo[:], wo[:], Act.Sqrt)
            nc.scalar.activation(wo[:], wo[:], Act.Sign)
            sre_sb = const.tile([128, FT * 4], f32)
            sim_sb = const.tile([128, FT * 4], f32)
            nc.vector.memset(sre_sb[:, (FT - 1) * 4:], 0.0)
            nc.vector.memset(sim_sb[:, (FT - 1) * 4:], 0.0)
            sA_sb = const.tile([128, FT * 4], f32)
            sB_sb = const.tile([128, FT * 4], f32)
            sC_sb = const.tile([128, FT * 4], f32)
            sD_sb = const.tile([128, FT * 4], f32)

            psy = psypool.tile([128, BPC * 64], f32, tag="psy")

            n_emitted = [0]
            N_CMM = 2 * FT  # total stage-C matmuls

            def emit_stage_c(ft, us):
                U, V = us
                nc.tensor.matmul(psy[:], cosa_sb[:],
                                 U[:].rearrange("p b s -> p (b s)"),
                                 start=(n_emitted[0] == 0), stop=False)
                nc.tensor.matmul(psy[:], nsina_sb[:],
                                 V[:].rearrange("p b s -> p (b s)"),
                                 start=False,
                                 stop=(n_emitted[0] == N_CMM - 2))
                n_emitted[0] += 2

            pend = []

            def _bcast(src, ft):
                return src[:, ft, :][:, None, :].broadcast_to([128, BPC, 64])

            def _sbcast(src, fsl):
                return src[:, fsl][:, :, None].broadcast_to([128, BPC, 64])

            # twiddle products on GpSimd (u = phi * S, broadcast both ways),
            # folded to U = u1-u2, V = v1+v2 so the PE only needs 2 IFFT
            # matmuls per tile.  Split in two halves: the re-products depend
            # only on S_re and are emitted mid-way through the DVE chain so
            # GpSimd overlaps the remaining stts (shortens the serial tail).
            def emit_uv_re(ft):
                fsl = slice(ft * 4, (ft + 1) * 4)
                u1 = uvpool.tile([128, BPC, 64], f32, tag="u1", name=f"u1_{ft}")
                v1 = uvpool.tile([128, BPC, 64], f32, tag="v1", name=f"v1_{ft}")
                nc.gpsimd.tensor_tensor(u1[:], _bcast(cphi_sb, ft),
                                        _sbcast(sre_sb, fsl), op=mult)
                nc.gpsimd.tensor_tensor(v1[:], _bcast(sphi_sb, ft),
                                        _sbcast(sre_sb, fsl), op=mult)
                return u1, v1

            def emit_uv_im(ft, u1, v1):
                # for the final tile the GpSimd chain is the serial tail;
                # the DVE is idle by then and ~1.7x faster per op
                eng = nc.vector if ft == FT - 2 else nc.gpsimd
                fsl = slice(ft * 4, (ft + 1) * 4)
                u2 = uvpool.tile([128, BPC, 64], f32, tag="u2", name=f"u2_{ft}")
                v2 = uvpool.tile([128, BPC, 64], f32, tag="v2", name=f"v2_{ft}")
                Ut = uvpool.tile([128, BPC, 64], bf16, tag="U", name=f"U_{ft}")
                Vt = uvpool.tile([128, BPC, 64], bf16, tag="V", name=f"V_{ft}")
                eng.tensor_tensor(u2[:], _bcast(sphi_sb, ft),
                                  _sbcast(sim_sb, fsl), op=mult)
                eng.tensor_tensor(v2[:], _bcast(cphi_sb, ft),
                                  _sbcast(sim_sb, fsl), op=mult)
                eng.tensor_tensor(Ut[:], u1[:], u2[:], op=subtract)
                eng.tensor_tensor(Vt[:], v1[:], v2[:], op=add)
                pend.append((ft, (Ut, Vt)))

            def emit_uv46():
                # k=4096 bin: S_im == 0 there, so U = cphi*S_re, V = sphi*S_re
                ft = FT - 1
                fsl = slice(ft * 4, (ft + 1) * 4)
                Ut = uvpool.tile([128, BPC, 64], bf16, tag="U", name="U_46")
                Vt = uvpool.tile([128, BPC, 64], bf16, tag="V", name="V_46")
                nc.gpsimd.tensor_tensor(Ut[:], _bcast(cphi_sb, ft),
                                        _sbcast(sre_sb, fsl), op=mult)
                nc.gpsimd.tensor_tensor(Vt[:], _bcast(sphi_sb, ft),
                                        _sbcast(sre_sb, fsl), op=mult)
                pend.append((ft, (Ut, Vt)))

            # k=4096 bin tile (ft == FT-1) runs first: it needs only x.
            for ft in [FT - 1] + list(range(FT - 1)):
                if ft == FT - 1:
                    # k=4096 bin: P1/P2 via 1-column matmuls onto partition 0.
                    # cphi row 4096 is (-1)^s/P (sphi row 0, cosa row 0 all-1,
                    # nsina row 0 all-0), so the generic twiddle+IFFT path
                    # below handles this bin exactly.
                    ps46 = {}
                    for hi, c0 in ((0, 0), (1, HT)):
                        for col in (0, 1):
                            ps = pstpool.tile([128, HT], f32, tag="pst",
                                              name=f"ps46_{col}_{hi}")
                            for ck in range(4):
                                nc.tensor.matmul(
                                    ps[0:1, :],
                                    a46_sb[:, ck, col:col + 1],
                                    x_sb[:, ck, c0:c0 + HT],
                                    start=(ck == 0),
                                    stop=(ck == 3),
                                )
                            ps46[(col, hi)] = ps
                    c46 = {}
                    for hi in (0, 1):
                        c_t = scr.tile([128, HT], f32, tag=f"c2{hi}",
                                       name=f"c46_{hi}")
                        nc.scalar.activation(c_t[0:1, :], ps46[(1, hi)][0:1, :],
                                             Act.Copy)
                        c46[hi] = c_t
                    for bl in range(BPC):
                        idx = ft * 4 + bl
                        hi, s0 = bl // 2, (bl % 2) * HW
                        seg = slice(s0, s0 + HW)
                        sc = scr.tile([128, HW], f32, tag="sc0",
                                      name=f"sc46_{bl}")
                        nc.vector.scalar_tensor_tensor(
                            sc[0:1, :], ps46[(0, hi)][0:1, seg], 1.0,
                            c46[hi][0:1, seg],
                            bypass, mult,
                            accum_out=sre_sb[0:1, idx:idx + 1])
                else:
                    a_t = apool.tile([128, 4, 512], bf16, tag="a")
                    nc.sync.dma_start(a_t[:], a_d[ft])
                if ft == 4:
                    # bulky twiddle constants: sync-queue position after a4
                    # so x/a0..a4 get the full HBM bandwidth first (GpSimd
                    # only needs them ~4 periods in).  The scheduler keeps
                    # same-queue DMA emission order; the Act queue does not.
                    # All uv emission for earlier tiles is deferred to here:
                    # a cphi/sphi read emitted before these dma_starts would
                    # order the DMA after the read and compute on garbage.
                    nc.sync.dma_start(cphi_sb[:], cphi_d)
                    nc.sync.dma_start(sphi_sb[:], sphi_d)
                    nc.sync.dma_start(cosa_sb[:], cosa_d)
                    nc.sync.dma_start(nsina_sb[:], nsina_d)
                    emit_uv46()
                    for f in range(4):
                        emit_uv_im(f, *emit_uv_re(f))
                psm = {}
                cpy = {}
                # components 2,3 first: Act evacuates each 1-bank half to
                # SBUF right after its 4 matmuls (the stt in1 operand); 0,1
                # stay in PSUM so each stt uses only one SBUF read port
                for m in (2, 3, 0, 1) if ft < FT - 1 else ():
                    msl = slice(m * 128, (m + 1) * 128)
                    for hi, c0 in ((0, 0), (1, HT)):
                        ps = pstpool.tile([128, HT], f32, tag="pst",
                                          name=f"ps{m}_{hi}_{ft}")
                        for ck in range(4):
                            nc.tensor.matmul(
                                ps[:, :],
                                a_t[:, ck, msl],
                                x_sb[:, ck, c0:c0 + HT],
                                start=(ck == 0),
                                stop=(ck == 3),
                            )
                        psm[(m, hi)] = ps
                        if m in (2, 3):
                            c_m = scr.tile([128, HT], f32, tag=f"c{m}{hi}",
                                           name=f"c{m}{hi}_{ft}")
                            nc.scalar.activation(c_m[:], ps[:], Act.Copy)
                            cpy[(m, hi)] = c_m

                # A=sum p0*p2, B=sum p1*p3, C=sum p0*p3, D=sum p1*p2.
                # A,B first so S_re = A-B is ready mid-chain and GpSimd's
                # re-products overlap the C,D stts.
                def stt_chain(tg, pa, cb, dst):
                    for bl in range(BPC):
                        idx = ft * 4 + bl
                        hi, s0 = bl // 2, (bl % 2) * HW
                        seg = slice(s0, s0 + HW)
                        sc = scr.tile([128, HW], f32, tag=tg,
                                      name=f"{tg}_{ft}_{bl}")
                        nc.vector.scalar_tensor_tensor(
                            sc[:], psm[(pa, hi)][:, seg], 1.0,
                            cpy[(cb, hi)][:, seg],
                            bypass, mult,
                            accum_out=dst[:, idx:idx + 1])

                fsl = slice(ft * 4, (ft + 1) * 4)
                if ft < FT - 1:
                    stt_chain("sc0", 0, 2, sA_sb)
                    stt_chain("sc1", 1, 3, sB_sb)
                    nc.vector.tensor_sub(sre_sb[:, fsl], sA_sb[:, fsl],
                                         sB_sb[:, fsl])
                    u1 = v1 = None
                    if ft >= 4:
                        u1, v1 = emit_uv_re(ft)
                    stt_chain("sc2", 0, 3, sC_sb)
                    stt_chain("sc3", 1, 2, sD_sb)
                    nc.vector.tensor_add(sim_sb[:, fsl], sC_sb[:, fsl],
                                         sD_sb[:, fsl])
                    if ft >= 4:
                        emit_uv_im(ft, u1, v1)

                # emit IFFT matmuls 3 iterations late to keep the PE queue fed
                while len(pend) > 3:
                    emit_stage_c(*pend.pop(0))
            while pend:
                emit_stage_c(*pend.pop(0))

            # ---- stage D: signed sqrt, per-batch l2 norm, store ----
            # bf16 |Y| feeds the norm matmul (1-pass bf16 instead of the
            # double-pass fp32 LOW_HIGH) and the Sqrt; error ~0.2% final
            absy = const.tile([128, BPC * 64], bf16)
            nc.scalar.activation(absy[:], psy[:], Act.Abs)
            sqy = const.tile([128, BPC * 64], f32)
            nc.scalar.activation(sqy[:], absy[:], Act.Sqrt)
            sgn = const.tile([128, BPC * 64], f32)
            nc.scalar.activation(sgn[:], psy[:], Act.Sign)
            ys = const.tile([128, BPC * 64], f32)
            nc.vector.tensor_mul(ys[:], sqy[:], sgn[:])

            # norm^2 per batch = sum_p y^2 = sum_p |Y|  (Y = pre-sqrt value)
            psn = pstpool.tile([128, BPC * 64], f32, tag="pst", name="psn")
            nc.tensor.matmul(psn[0:1, :], ones_sb[:], absy[:],
                             start=True, stop=True)
            nsq = const.tile([1, BPC], f32)
            nc.vector.reduce_sum(
                out=nsq[:],
                in_=psn[0:1, :].rearrange("p (b s) -> p b s", b=BPC),
                axis=mybir.AxisListType.X,
            )
            nc.vector.tensor_scalar_max(nsq[:], nsq[:], 1e-10)
            sqn = const.tile([1, BPC], f32)
            nc.scalar.activation(sqn[:], nsq[:], Act.Sqrt)
            invn = const.tile([1, BPC], f32)
            nc.vector.reciprocal(invn[:], sqn[:])
            invn16 = const.tile([1, BPC], bf16)
            nc.scalar.activation(invn16[:], invn[:], Act.Copy)

            onesrow = const.tile([1, 128], bf16)
            nc.vector.memset(onesrow[:], 1.0)
            psb = pstpool.tile([128, BPC], f32, tag="pst", name="psb")
            nc.tensor.matmul(psb[:, 0:BPC], onesrow[0:1, :], invn16[0:1, :],
                             start=True, stop=True)
            inv_b = psb[:, 0:BPC][:, :, None].broadcast_to([128, BPC, 64])
            fin = const.tile([128, BPC * 64], f32)
            nc.vector.tensor_tensor(
                fin[:].rearrange("p (b s) -> p b s", b=BPC),
                ys[:].rearrange("p (b s) -> p b s", b=BPC),
                inv_b,
                op=mult,
            )
            # single descriptor for all 4 batch rows
            nc.sync.dma_start(
                y_d.rearrange("b (q s) -> q b s", q=128),
                fin[:].rearrange("p (b s) -> p b s", b=BPC),
            )

    nc.compile()
    return nc


def _to_bf16(a):
    import ml_dtypes
    return np.asarray(a, np.float32).astype(ml_dtypes.bfloat16)


def _host_prep(x, M1, M2):
    x = np.ascontiguousarray(np.asarray(x, np.float32))
    M1 = np.asarray(M1, np.float32)
    M2 = np.asarray(M2, np.float32)

    h1 = np.argmax(np.abs(M1), axis=1)
    s1 = M1[np.arange(C), h1].astype(np.float64)
    h2 = np.argmax(np.abs(M2), axis=1)
    s2 = M2[np.arange(C), h2].astype(np.float64)

    k = np.arange(NSLOT, dtype=np.float64)
    valid = k <= P // 2
    ang1 = 2 * np.pi * np.outer(h1.astype(np.float64), k) / P
    ang2 = 2 * np.pi * np.outer(h2.astype(np.float64), k) / P
    # a[ft, c, m*128 + j]: m in (A1re, A1im, A2re, A2im), freq = ft*128 + j
    a = np.empty((FT - 1, C, 512), np.float32)
    a1re = (s1[:, None] * np.cos(ang1) * valid).astype(np.float32)
    a1im = (-s1[:, None] * np.sin(ang1) * valid).astype(np.float32)
    a2re = (s2[:, None] * np.cos(ang2) * valid).astype(np.float32)
    a2im = (-s2[:, None] * np.sin(ang2) * valid).astype(np.float32)
    for ft in range(FT - 1):
        ksl = slice(ft * 128, (ft + 1) * 128)
        a[ft, :, 0:128] = a1re[:, ksl]
        a[ft, :, 128:256] = a1im[:, ksl]
        a[ft, :, 256:384] = a2re[:, ksl]
        a[ft, :, 384:512] = a2im[:, ksl]

    w = np.where(valid, 2.0 / P, 0.0)
    w[0] = 1.0 / P
    w[P // 2] = 1.0 / P
    s_idx = np.arange(64, dtype=np.float64)
    phi = 2 * np.pi * np.outer(k, s_idx) / P
    cphi = (w[:, None] * np.cos(phi)).astype(np.float32).reshape(FT, 128, 64)
    sphi = (w[:, None] * np.sin(phi)).astype(np.float32).reshape(FT, 128, 64)

    km = np.arange(128, dtype=np.float64)
    alpha = 2 * np.pi * np.outer(km, km) / 128
    cosa = np.cos(alpha).astype(np.float32)
    nsina = (-np.sin(alpha)).astype(np.float32)

    xt = np.ascontiguousarray(x.reshape(B * HW, C).T)  # [C, 6272]

    # pre-transposed, DMA-contiguous layouts
    a_t = np.ascontiguousarray(
        a.reshape(FT - 1, 4, 128, 512).transpose(0, 2, 1, 3))
    # k=4096 columns: a46[p, ck, 0/1] = s1/2[c] * cos(pi*h1/2[c]), c=ck*128+p
    a46 = np.stack([a1re[:, 4096], a2re[:, 4096]], axis=1)  # [C, 2]
    a46 = np.ascontiguousarray(a46.reshape(4, 128, 2).transpose(1, 0, 2))
    cphi_t = np.ascontiguousarray(cphi.transpose(1, 0, 2))  # [128p,FT,64]
    sphi_t = np.ascontiguousarray(sphi.transpose(1, 0, 2))
    xt_t = xt.reshape(4, 128, B * HW).transpose(1, 0, 2)    # [128p,4ck,T]
    return (_to_bf16(a_t), _to_bf16(a46), cphi_t, sphi_t, _to_bf16(cosa),
            _to_bf16(nsina), _to_bf16(xt_t))


def _make_in_maps(x, M1, M2):
    a, a46, cphi, sphi, cosa, nsina, xt = _host_prep(x, M1, M2)
    in_maps = []
    for r in range(NCORES):
        in_maps.append({
            "a": a,
            "a46": a46,
            "x": np.ascontiguousarray(xt[:, :, r * T:(r + 1) * T]),
            "cphi": cphi,
            "sphi": sphi,
            "cosa": cosa,
            "nsina": nsina,
        })
    return in_maps


def kernel(x, M1, M2):
    from concourse.bass_utils import run_bass_kernel_spmd

    if "nc" not in _CACHE:
        _CACHE["nc"] = _build_program()
    nc = _CACHE["nc"]

    in_maps = _make_in_maps(x, M1, M2)
    res = run_bass_kernel_spmd(nc, in_maps, core_ids=list(range(NCORES)))
    out = np.concatenate([res.results[r]["y"] for r in range(NCORES)], axis=0)
    return out.astype(np.float32)


# revision 36
# speedup vs baseline: 1.2981x; 1.0008x over previous
"""Trainium2 kernel for CompactBilinearLayer (count-sketch bilinear pooling).

Math: reference computes y = l2norm(signed_sqrt(sum_hw Re IFFT(FFT(x@M1)*FFT(x@M2)))).
Since M1/M2 are count-sketch matrices (one +-1 per row), FFT(x@M1) == x @ A1 with
A1[c,k] = s1[c] * exp(-2pi i h1[c] k / P) — a dense [512, K] matrix computable on the
host from M1 in O(C*K). The IFFT is linear, so the spatial sum moves before it.
Hermitian symmetry means only k = 0..4096 are needed.  Per core (4 batch elements,
784 spatial positions — fully batch-local, no collectives):
  A: P1/P2 projections = A^T @ x^T, single-pass bf16 matmuls; per-component
     pairs of 1-bank [128,392] PSUM tiles (7-deep ring) so Act evacuation
     copies start right after each half-group and never gate the next tile
  B: S[k,b] = sum_t (P1*P2) per batch via fused DVE scalar_tensor_tensor
     (product+reduce in one op), one PSUM + one SBUF operand each.  A,B
     chains run before C,D so S_re is ready mid-chain and GpSimd's
     re-products overlap the remaining stts.
  C: IFFT via two-step factorization n=64q+s: GpSimd computes the twiddle
     products and folds them to U = u1-u2, V = v1+v2 (bf16), PE accumulates
     just 2 bf16 matmuls over k%128 into psy.  Stage-C matmuls are emitted
     3 iterations late so the in-order PE queue never waits on the
     DVE->GpSimd chain; the final tile's fold runs on the (by then idle) DVE.
  D: signed sqrt + per-batch L2 norm (all-partition replicated column sums,
     no broadcast matmul) + two half stores so the first transfer overlaps
     the second half's scale.
The k=4096 bin tile runs FIRST (it only needs x, not the streamed A tiles),
overlapping the startup DMA window.  The 13-deep A-tile pool gives the
single Sync DGE queue enough issue lead; the bulky twiddle constants are
queued behind a0..a4 so x and the first A tiles get full HBM bandwidth
(their consumers, uv products for tiles 0..3, are deferred past those
dma_starts — a cphi read emitted earlier would order the DMA after it).
"""
import numpy as np

P = 8192
C = 512
FT = 33            # frequency tiles of 128 -> 4224 slots >= 4097
NSLOT = FT * 128
NCORES = 8
BPC = 4            # batch elems per core
HW = 196           # spatial positions per batch elem
T = BPC * HW       # 784 positions per core
HT = T // 2        # 392, one PSUM bank of f32
B = 32

_CACHE = {}


def _build_program():
    import concourse.bass as bass
    import concourse.tile as tile
    from concourse import bacc, mybir

    f32 = mybir.dt.float32
    f32r = mybir.dt.float32r
    bf16 = mybir.dt.bfloat16
    nc = bacc.Bacc("TRN2", target_bir_lowering=False, debug=False,
                   num_devices=NCORES)

    a_d = nc.dram_tensor("a", [FT - 1, 128, 4, 512], bf16, kind="ExternalInput").ap()
    a46_d = nc.dram_tensor("a46", [128, 4, 2], bf16, kind="ExternalInput").ap()
    x_d = nc.dram_tensor("x", [128, 4, T], bf16, kind="ExternalInput").ap()
    cphi_d = nc.dram_tensor("cphi", [128, FT, 64], f32, kind="ExternalInput").ap()
    sphi_d = nc.dram_tensor("sphi", [128, FT, 64], f32, kind="ExternalInput").ap()
    cosa_d = nc.dram_tensor("cosa", [128, 128], bf16, kind="ExternalInput").ap()
    nsina_d = nc.dram_tensor("nsina", [128, 128], bf16, kind="ExternalInput").ap()
    y_d = nc.dram_tensor("y", [BPC, P], f32, kind="ExternalOutput").ap()

    mult = mybir.AluOpType.mult
    subtract = mybir.AluOpType.subtract
    add = mybir.AluOpType.add
    bypass = mybir.AluOpType.bypass
    Act = mybir.ActivationFunctionType

    with tile.TileContext(nc) as tc:
        with (
            tc.tile_pool(name="const", bufs=1) as const,
            tc.tile_pool(name="apool", bufs=13) as apool,
            tc.tile_pool(name="pst", bufs=7, space="PSUM") as pstpool,
            tc.tile_pool(name="psyp", bufs=1, space="PSUM") as psypool,
            tc.tile_pool(name="scr", bufs=4) as scr,
            tc.tile_pool(name="uv", bufs=7) as uvpool,
        ):
            a46_sb = const.tile([128, 4, 2], bf16)
            nc.sync.dma_start(a46_sb[:], a46_d)
            x_sb = const.tile([128, 4, T], bf16)
            # two descriptors: the first ft46 matmuls only touch t < HT,
            # so they can start ~1us before the second half lands
            nc.sync.dma_start(x_sb[:, :, 0:HT], x_d[:, :, 0:HT])
            nc.sync.dma_start(x_sb[:, :, HT:T], x_d[:, :, HT:T])
            cphi_sb = const.tile([128, FT, 64], f32)
            sphi_sb = const.tile([128, FT, 64], f32)
            cosa_sb = const.tile([128, 128], bf16)
            nsina_sb = const.tile([128, 128], bf16)
            ones_sb = const.tile([128, 128], bf16)
            nc.vector.memset(ones_sb[:], 1.0)
            # preload the Abs/Sqrt/Sign activation tables during the initial
            # DMA window so stage D doesn't pay the ~2.6us table switch
            warm = const.tile([1, 1], f32)
            nc.vector.memset(warm[:], 1.0)
            wo = const.tile([1, 1], f32)
            nc.scalar.activation(wo[:], warm[:], Act.Abs)
            nc.scalar.activation(wo[:], wo[:], Act.Sqrt)
            nc.scalar.activation(wo[:], wo[:], Act.Sign)
            sre_sb = const.tile([128, FT * 4], f32)
            sim_sb = const.tile([128, FT * 4], f32)
            nc.vector.memset(sre_sb[:, (FT - 1) * 4:], 0.0)
            nc.vector.memset(sim_sb[:, (FT - 1) * 4:], 0.0)
            sA_sb = const.tile([128, FT * 4], f32)
            sB_sb = const.tile([128, FT * 4], f32)
            sC_sb = const.tile([128, FT * 4], f32)
            sD_sb = const.tile([128, FT * 4], f32)

            psy = psypool.tile([128, BPC * 64], f32, tag="psy")

            n_emitted = [0]
            N_CMM = 2 * FT  # total stage-C matmuls

            def emit_stage_c(ft, us):
                U, V = us
                nc.tensor.matmul(psy[:], cosa_sb[:],
                                 U[:].rearrange("p b s -> p (b s)"),
                                 start=(n_emitted[0] == 0), stop=False)
                nc.tensor.matmul(psy[:], nsina_sb[:],
                                 V[:].rearrange("p b s -> p (b s)"),
                                 start=False,
                                 stop=(n_emitted[0] == N_CMM - 2))
                n_emitted[0] += 2

            pend = []

            def _bcast(src, ft):
                return src[:, ft, :][:, None, :].broadcast_to([128, BPC, 64])

            def _sbcast(src, fsl):
                return src[:, fsl][:, :, None].broadcast_to([128, BPC, 64])

            # twiddle products on GpSimd (u = phi * S, broadcast both ways),
            # folded to U = u1-u2, V = v1+v2 so the PE only needs 2 IFFT
            # matmuls per tile.  Split in two halves: the re-products depend
            # only on S_re and are emitted mid-way through the DVE chain so
            # GpSimd overlaps the remaining stts (shortens the serial tail).
            def emit_uv_re(ft):
                fsl = slice(ft * 4, (ft + 1) * 4)
                u1 = uvpool.tile([128, BPC, 64], f32, tag="u1", name=f"u1_{ft}")
                v1 = uvpool.tile([128, BPC, 64], f32, tag="v1", name=f"v1_{ft}")
                nc.gpsimd.tensor_tensor(u1[:], _bcast(cphi_sb, ft),
                                        _sbcast(sre_sb, fsl), op=mult)
                nc.gpsimd.tensor_tensor(v1[:], _bcast(sphi_sb, ft),
                                        _sbcast(sre_sb, fsl), op=mult)
                return u1, v1

            def emit_uv_im(ft, u1, v1):
                # for the final tile the GpSimd chain is the serial tail;
                # the DVE is idle by then and ~1.7x faster per op
                eng = nc.vector if ft == FT - 2 else nc.gpsimd
                fsl = slice(ft * 4, (ft + 1) * 4)
                u2 = uvpool.tile([128, BPC, 64], f32, tag="u2", name=f"u2_{ft}")
                v2 = uvpool.tile([128, BPC, 64], f32, tag="v2", name=f"v2_{ft}")
                Ut = uvpool.tile([128, BPC, 64], bf16, tag="U", name=f"U_{ft}")
                Vt = uvpool.tile([128, BPC, 64], bf16, tag="V", name=f"V_{ft}")
                eng.tensor_tensor(u2[:], _bcast(sphi_sb, ft),
                                  _sbcast(sim_sb, fsl), op=mult)
                eng.tensor_tensor(v2[:], _bcast(cphi_sb, ft),
                                  _sbcast(sim_sb, fsl), op=mult)
                eng.tensor_tensor(Ut[:], u1[:], u2[:], op=subtract)
                eng.tensor_tensor(Vt[:], v1[:], v2[:], op=add)
                pend.append((ft, (Ut, Vt)))

            def emit_uv46():
                # k=4096 bin: S_im == 0 there, so U = cphi*S_re, V = sphi*S_re
                ft = FT - 1
                fsl = slice(ft * 4, (ft + 1) * 4)
                Ut = uvpool.tile([128, BPC, 64], bf16, tag="U", name="U_46")
                Vt = uvpool.tile([128, BPC, 64], bf16, tag="V", name="V_46")
                nc.gpsimd.tensor_tensor(Ut[:], _bcast(cphi_sb, ft),
                                        _sbcast(sre_sb, fsl), op=mult)
                nc.gpsimd.tensor_tensor(Vt[:], _bcast(sphi_sb, ft),
                                        _sbcast(sre_sb, fsl), op=mult)
                pend.append((ft, (Ut, Vt)))

            # k=4096 bin tile (ft == FT-1) runs first: it needs only x.
            for ft in [FT - 1] + list(range(FT - 1)):
                if ft == FT - 1:
                    # k=4096 bin: P1/P2 via 1-column matmuls onto partition 0.
                    # cphi row 4096 is (-1)^s/P (sphi row 0, cosa row 0 all-1,
                    # nsina row 0 all-0), so the generic twiddle+IFFT path
                    # below handles this bin exactly.
                    ps46 = {}
                    for hi, c0 in ((0, 0), (1, HT)):
                        for col in (0, 1):
                            ps = pstpool.tile([128, HT], f32, tag="pst",
                                              name=f"ps46_{col}_{hi}")
                            for ck in range(4):
                                nc.tensor.matmul(
                                    ps[0:1, :],
                                    a46_sb[:, ck, col:col + 1],
                                    x_sb[:, ck, c0:c0 + HT],
                                    start=(ck == 0),
                                    stop=(ck == 3),
                                )
                            ps46[(col, hi)] = ps
                    c46 = {}
                    for hi in (0, 1):
                        c_t = scr.tile([128, HT], f32, tag=f"c2{hi}",
                                       name=f"c46_{hi}")
                        nc.scalar.activation(c_t[0:1, :], ps46[(1, hi)][0:1, :],
                                             Act.Copy)
                        c46[hi] = c_t
                    for bl in range(BPC):
                        idx = ft * 4 + bl
                        hi, s0 = bl // 2, (bl % 2) * HW
                        seg = slice(s0, s0 + HW)
                        sc = scr.tile([128, HW], f32, tag="sc0",
                                      name=f"sc46_{bl}")
                        nc.vector.scalar_tensor_tensor(
                            sc[0:1, :], ps46[(0, hi)][0:1, seg], 1.0,
                            c46[hi][0:1, seg],
                            bypass, mult,
                            accum_out=sre_sb[0:1, idx:idx + 1])
                else:
                    a_t = apool.tile([128, 4, 512], bf16, tag="a")
                    nc.sync.dma_start(a_t[:], a_d[ft])
                if ft == 4:
                    # bulky twiddle constants: sync-queue position after a4
                    # so x/a0..a4 get the full HBM bandwidth first (GpSimd
                    # only needs them ~4 periods in).  The scheduler keeps
                    # same-queue DMA emission order; the Act queue does not.
                    # All uv emission for earlier tiles is deferred to here:
                    # a cphi/sphi read emitted before these dma_starts would
                    # order the DMA after the read and compute on garbage.
                    nc.sync.dma_start(cphi_sb[:], cphi_d)
                    nc.sync.dma_start(sphi_sb[:], sphi_d)
                    nc.sync.dma_start(cosa_sb[:], cosa_d)
                    nc.sync.dma_start(nsina_sb[:], nsina_d)
                    emit_uv46()
                    for f in range(4):
                        emit_uv_im(f, *emit_uv_re(f))
                psm = {}
                cpy = {}
                # components 2,3 first: Act evacuates each 1-bank half to
                # SBUF right after its 4 matmuls (the stt in1 operand); 0,1
                # stay in PSUM so each stt uses only one SBUF read port
                for m in (2, 3, 0, 1) if ft < FT - 1 else ():
                    msl = slice(m * 128, (m + 1) * 128)
                    for hi, c0 in ((0, 0), (1, HT)):
                        ps = pstpool.tile([128, HT], f32, tag="pst",
                                          name=f"ps{m}_{hi}_{ft}")
                        for ck in range(4):
                            nc.tensor.matmul(
                                ps[:, :],
                                a_t[:, ck, msl],
                                x_sb[:, ck, c0:c0 + HT],
                                start=(ck == 0),
                                stop=(ck == 3),
                            )
                        psm[(m, hi)] = ps
                        if m in (2, 3):
                            c_m = scr.tile([128, HT], f32, tag=f"c{m}{hi}",
                                           name=f"c{m}{hi}_{ft}")
                            nc.scalar.activation(c_m[:], ps[:], Act.Copy)
                            cpy[(m, hi)] = c_m

                # A=sum p0*p2, B=sum p1*p3, C=sum p0*p3, D=sum p1*p2.
                # A,B first so S_re = A-B is ready mid-chain and GpSimd's
                # re-products overlap the C,D stts.
                def stt_chain(tg, pa, cb, dst):
                    for bl in range(BPC):
                        idx = ft * 4 + bl
                        hi, s0 = bl // 2, (bl % 2) * HW
                        seg = slice(s0, s0 + HW)
                        sc = scr.tile([128, HW], f32, tag=tg,
                                      name=f"{tg}_{ft}_{bl}")
                        nc.vector.scalar_tensor_tensor(
                            sc[:], psm[(pa, hi)][:, seg], 1.0,
                            cpy[(cb, hi)][:, seg],
                            bypass, mult,
                            accum_out=dst[:, idx:idx + 1])

                fsl = slice(ft * 4, (ft + 1) * 4)
                if ft < FT - 1:
                    stt_chain("sc0", 0, 2, sA_sb)
                    stt_chain("sc1", 1, 3, sB_sb)
                    nc.vector.tensor_sub(sre_sb[:, fsl], sA_sb[:, fsl],
                                         sB_sb[:, fsl])
                    u1 = v1 = None
                    if ft >= 4:
                        u1, v1 = emit_uv_re(ft)
                    stt_chain("sc2", 0, 3, sC_sb)
                    stt_chain("sc3", 1, 2, sD_sb)
                    nc.vector.tensor_add(sim_sb[:, fsl], sC_sb[:, fsl],
                                         sD_sb[:, fsl])
                    if ft >= 4:
                        emit_uv_im(ft, u1, v1)

                # emit IFFT matmuls 3 iterations late to keep the PE queue fed
                while len(pend) > 3:
                    emit_stage_c(*pend.pop(0))
            while pend:
                emit_stage_c(*pend.pop(0))

            # ---- stage D: signed sqrt, per-batch l2 norm, store ----
            # bf16 |Y| feeds the norm matmul (1-pass bf16 instead of the
            # double-pass fp32 LOW_HIGH) and the Sqrt; error ~0.2% final
            absy = const.tile([128, BPC * 64], bf16)
            nc.scalar.activation(absy[:], psy[:], Act.Abs)
            sqy = const.tile([128, BPC * 64], f32)
            nc.scalar.activation(sqy[:], absy[:], Act.Sqrt)
            sgn = const.tile([128, BPC * 64], f32)
            nc.scalar.activation(sgn[:], psy[:], Act.Sign)
            ys = const.tile([128, BPC * 64], f32)
            nc.vector.tensor_mul(ys[:], sqy[:], sgn[:])

            # norm^2 per batch = sum_p y^2 = sum_p |Y|  (Y = pre-sqrt value).
            # All-ones [128,128] stationary replicates the column sums to
            # every partition, so no extra broadcast matmul is needed for
            # the final scale.
            psn = pstpool.tile([128, BPC * 64], f32, tag="pst", name="psn")
            nc.tensor.matmul(psn[:, :], ones_sb[:], absy[:],
                             start=True, stop=True)
            nsq = const.tile([128, BPC], f32)
            nc.vector.reduce_sum(
                out=nsq[:],
                in_=psn[:, :].rearrange("p (b s) -> p b s", b=BPC),
                axis=mybir.AxisListType.X,
            )
            nc.vector.tensor_scalar_max(nsq[:], nsq[:], 1e-10)
            sqn = const.tile([128, BPC], f32)
            nc.scalar.activation(sqn[:], nsq[:], Act.Sqrt)
            invn = const.tile([128, BPC], f32)
            nc.vector.reciprocal(invn[:], sqn[:])

            inv_b = invn[:, :, None].broadcast_to([128, BPC, 64])
            fin = const.tile([128, BPC * 64], f32)
            # two halves: the first store's transfer overlaps the second
            # half's scale + descriptor write
            for h0 in (0, 2):
                bs = slice(h0 * 64, (h0 + 2) * 64)
                nc.vector.tensor_tensor(
                    fin[:, bs].rearrange("p (b s) -> p b s", b=2),
                    ys[:, bs].rearrange("p (b s) -> p b s", b=2),
                    inv_b[:, h0:h0 + 2, :],
                    op=mult,
                )
                nc.sync.dma_start(
                    y_d[h0:h0 + 2].rearrange("b (q s) -> q b s", q=128),
                    fin[:, bs].rearrange("p (b s) -> p b s", b=2),
                )

    nc.compile()
    return nc


def _to_bf16(a):
    import ml_dtypes
    return np.asarray(a, np.float32).astype(ml_dtypes.bfloat16)


def _host_prep(x, M1, M2):
    x = np.ascontiguousarray(np.asarray(x, np.float32))
    M1 = np.asarray(M1, np.float32)
    M2 = np.asarray(M2, np.float32)

    h1 = np.argmax(np.abs(M1), axis=1)
    s1 = M1[np.arange(C), h1].astype(np.float64)
    h2 = np.argmax(np.abs(M2), axis=1)
    s2 = M2[np.arange(C), h2].astype(np.float64)

    k = np.arange(NSLOT, dtype=np.float64)
    valid = k <= P // 2
    ang1 = 2 * np.pi * np.outer(h1.astype(np.float64), k) / P
    ang2 = 2 * np.pi * np.outer(h2.astype(np.float64), k) / P
    # a[ft, c, m*128 + j]: m in (A1re, A1im, A2re, A2im), freq = ft*128 + j
    a = np.empty((FT - 1, C, 512), np.float32)
    a1re = (s1[:, None] * np.cos(ang1) * valid).astype(np.float32)
    a1im = (-s1[:, None] * np.sin(ang1) * valid).astype(np.float32)
    a2re = (s2[:, None] * np.cos(ang2) * valid).astype(np.float32)
    a2im = (-s2[:, None] * np.sin(ang2) * valid).astype(np.float32)
    for ft in range(FT - 1):
        ksl = slice(ft * 128, (ft + 1) * 128)
        a[ft, :, 0:128] = a1re[:, ksl]
        a[ft, :, 128:256] = a1im[:, ksl]
        a[ft, :, 256:384] = a2re[:, ksl]
        a[ft, :, 384:512] = a2im[:, ksl]

    w = np.where(valid, 2.0 / P, 0.0)
    w[0] = 1.0 / P
    w[P // 2] = 1.0 / P
    s_idx = np.arange(64, dtype=np.float64)
    phi = 2 * np.pi * np.outer(k, s_idx) / P
    cphi = (w[:, None] * np.cos(phi)).astype(np.float32).reshape(FT, 128, 64)
    sphi = (w[:, None] * np.sin(phi)).astype(np.float32).reshape(FT, 128, 64)

    km = np.arange(128, dtype=np.float64)
    alpha = 2 * np.pi * np.outer(km, km) / 128
    cosa = np.cos(alpha).astype(np.float32)
    nsina = (-np.sin(alpha)).astype(np.float32)

    xt = np.ascontiguousarray(x.reshape(B * HW, C).T)  # [C, 6272]

    # pre-transposed, DMA-contiguous layouts
    a_t = np.ascontiguousarray(
        a.reshape(FT - 1, 4, 128, 512).transpose(0, 2, 1, 3))
    # k=4096 columns: a46[p, ck, 0/1] = s1/2[c] * cos(pi*h1/2[c]), c=ck*128+p
    a46 = np.stack([a1re[:, 4096], a2re[:, 4096]], axis=1)  # [C, 2]
    a46 = np.ascontiguousarray(a46.reshape(4, 128, 2).transpose(1, 0, 2))
    cphi_t = np.ascontiguousarray(cphi.transpose(1, 0, 2))  # [128p,FT,64]
    sphi_t = np.ascontiguousarray(sphi.transpose(1, 0, 2))
    xt_t = xt.reshape(4, 128, B * HW).transpose(1, 0, 2)    # [128p,4ck,T]
    return (_to_bf16(a_t), _to_bf16(a46), cphi_t, sphi_t, _to_bf16(cosa),
            _to_bf16(nsina), _to_bf16(xt_t))


def _make_in_maps(x, M1, M2):
    a, a46, cphi, sphi, cosa, nsina, xt = _host_prep(x, M1, M2)
    in_maps = []
    for r in range(NCORES):
        in_maps.append({
            "a": a,
            "a46": a46,
            "x": np.ascontiguousarray(xt[:, :, r * T:(r + 1) * T]),
            "cphi": cphi,
            "sphi": sphi,
            "cosa": cosa,
            "nsina": nsina,
        })
    return in_maps


def kernel(x, M1, M2):
    from concourse.bass_utils import run_bass_kernel_spmd

    if "nc" not in _CACHE:
        _CACHE["nc"] = _build_program()
    nc = _CACHE["nc"]

    in_maps = _make_in_maps(x, M1, M2)
    res = run_bass_kernel_spmd(nc, in_maps, core_ids=list(range(NCORES)))
    out = np.concatenate([res.results[r]["y"] for r in range(NCORES)], axis=0)
    return out.astype(np.float32)


# revision 38
# speedup vs baseline: 1.3013x; 1.0025x over previous
"""Trainium2 kernel for CompactBilinearLayer (count-sketch bilinear pooling).

Math: reference computes y = l2norm(signed_sqrt(sum_hw Re IFFT(FFT(x@M1)*FFT(x@M2)))).
Since M1/M2 are count-sketch matrices (one +-1 per row), FFT(x@M1) == x @ A1 with
A1[c,k] = s1[c] * exp(-2pi i h1[c] k / P) — a dense [512, K] matrix computable on the
host from M1 in O(C*K). The IFFT is linear, so the spatial sum moves before it.
Hermitian symmetry means only k = 0..4096 are needed.  Per core (4 batch elements,
784 spatial positions — fully batch-local, no collectives):
  A: P1/P2 projections = A^T @ x^T, single-pass bf16 matmuls; per-component
     pairs of 1-bank [128,392] PSUM tiles (7-deep ring) so Act evacuation
     copies start right after each half-group and never gate the next tile
  B: S[k,b] = sum_t (P1*P2) per batch via fused DVE scalar_tensor_tensor
     (product+reduce in one op), one PSUM + one SBUF operand each.  A,B
     chains run before C,D so S_re is ready mid-chain and GpSimd's
     re-products overlap the remaining stts.
  C: IFFT via two-step factorization n=64q+s: GpSimd computes the twiddle
     products and folds them to U = u1-u2, V = v1+v2 (bf16), PE accumulates
     just 2 bf16 matmuls over k%128 into psy.  Stage-C matmuls are emitted
     3 iterations late so the in-order PE queue never waits on the
     DVE->GpSimd chain; the final tile's fold runs on the (by then idle) DVE.
  D: signed sqrt + per-batch L2 norm (all-partition replicated column sums,
     no broadcast matmul) + two half stores so the first transfer overlaps
     the second half's scale.
The k=4096 bin tile runs FIRST (it only needs x, not the streamed A tiles),
overlapping the startup DMA window.  The 13-deep A-tile pool gives the
single Sync DGE queue enough issue lead; the bulky twiddle constants are
queued behind a0..a4 so x and the first A tiles get full HBM bandwidth
(their consumers, uv products for tiles 0..3, are deferred past those
dma_starts — a cphi read emitted earlier would order the DMA after it).
"""
import numpy as np

P = 8192
C = 512
FT = 33            # frequency tiles of 128 -> 4224 slots >= 4097
NSLOT = FT * 128
NCORES = 8
BPC = 4            # batch elems per core
HW = 196           # spatial positions per batch elem
T = BPC * HW       # 784 positions per core
HT = T // 2        # 392, one PSUM bank of f32
B = 32

_CACHE = {}


def _build_program():
    import concourse.bass as bass
    import concourse.tile as tile
    from concourse import bacc, mybir

    f32 = mybir.dt.float32
    f32r = mybir.dt.float32r
    bf16 = mybir.dt.bfloat16
    nc = bacc.Bacc("TRN2", target_bir_lowering=False, debug=False,
                   num_devices=NCORES)

    a_d = nc.dram_tensor("a", [FT - 1, 128, 4, 512], bf16, kind="ExternalInput").ap()
    a46_d = nc.dram_tensor("a46", [128, 4, 2], bf16, kind="ExternalInput").ap()
    x_d = nc.dram_tensor("x", [128, 4, T], bf16, kind="ExternalInput").ap()
    cphi_d = nc.dram_tensor("cphi", [128, FT, 64], f32, kind="ExternalInput").ap()
    sphi_d = nc.dram_tensor("sphi", [128, FT, 64], f32, kind="ExternalInput").ap()
    cosa_d = nc.dram_tensor("cosa", [128, 128], bf16, kind="ExternalInput").ap()
    nsina_d = nc.dram_tensor("nsina", [128, 128], bf16, kind="ExternalInput").ap()
    y_d = nc.dram_tensor("y", [BPC, P], f32, kind="ExternalOutput").ap()

    mult = mybir.AluOpType.mult
    subtract = mybir.AluOpType.subtract
    add = mybir.AluOpType.add
    bypass = mybir.AluOpType.bypass
    Act = mybir.ActivationFunctionType

    with tile.TileContext(nc) as tc:
        with (
            tc.tile_pool(name="const", bufs=1) as const,
            tc.tile_pool(name="apool", bufs=13) as apool,
            tc.tile_pool(name="pst", bufs=7, space="PSUM") as pstpool,
            tc.tile_pool(name="psyp", bufs=1, space="PSUM") as psypool,
            tc.tile_pool(name="scr", bufs=4) as scr,
            tc.tile_pool(name="uv", bufs=7) as uvpool,
        ):
            a46_sb = const.tile([128, 4, 2], bf16)
            nc.sync.dma_start(a46_sb[:], a46_d)
            x_sb = const.tile([128, 4, T], bf16)
            # two descriptors: the first ft46 matmuls only touch t < HT,
            # so they can start ~1us before the second half lands
            nc.sync.dma_start(x_sb[:, :, 0:HT], x_d[:, :, 0:HT])
            nc.sync.dma_start(x_sb[:, :, HT:T], x_d[:, :, HT:T])
            cphi_sb = const.tile([128, FT, 64], f32)
            sphi_sb = const.tile([128, FT, 64], f32)
            cosa_sb = const.tile([128, 128], bf16)
            nsina_sb = const.tile([128, 128], bf16)
            ones_sb = const.tile([128, 128], bf16)
            nc.vector.memset(ones_sb[:], 1.0)
            # preload the Abs/Sqrt/Sign activation tables during the initial
            # DMA window so stage D doesn't pay the ~2.6us table switch
            warm = const.tile([1, 1], f32)
            nc.vector.memset(warm[:], 1.0)
            wo = const.tile([1, 1], f32)
            nc.scalar.activation(wo[:], warm[:], Act.Abs)
            nc.scalar.activation(wo[:], wo[:], Act.Sqrt)
            nc.scalar.activation(wo[:], wo[:], Act.Sign)
            sre_sb = const.tile([128, FT * 4], f32)
            sim_sb = const.tile([128, FT * 4], f32)
            nc.vector.memset(sre_sb[:, (FT - 1) * 4:], 0.0)
            nc.vector.memset(sim_sb[:, (FT - 1) * 4:], 0.0)
            sA_sb = const.tile([128, FT * 4], f32)
            sB_sb = const.tile([128, FT * 4], f32)
            sC_sb = const.tile([128, FT * 4], f32)
            sD_sb = const.tile([128, FT * 4], f32)

            psy = psypool.tile([128, BPC * 64], f32, tag="psy")

            n_emitted = [0]
            N_CMM = 2 * FT  # total stage-C matmuls

            def emit_stage_c(ft, us):
                U, V = us
                nc.tensor.matmul(psy[:], cosa_sb[:],
                                 U[:].rearrange("p b s -> p (b s)"),
                                 start=(n_emitted[0] == 0), stop=False)
                nc.tensor.matmul(psy[:], nsina_sb[:],
                                 V[:].rearrange("p b s -> p (b s)"),
                                 start=False,
                                 stop=(n_emitted[0] == N_CMM - 2))
                n_emitted[0] += 2

            pend = []

            def _bcast(src, ft):
                return src[:, ft, :][:, None, :].broadcast_to([128, BPC, 64])

            def _sbcast(src, fsl):
                return src[:, fsl][:, :, None].broadcast_to([128, BPC, 64])

            # twiddle products on GpSimd (u = phi * S, broadcast both ways),
            # folded to U = u1-u2, V = v1+v2 so the PE only needs 2 IFFT
            # matmuls per tile.  Split in two halves: the re-products depend
            # only on S_re and are emitted mid-way through the DVE chain so
            # GpSimd overlaps the remaining stts (shortens the serial tail).
            def emit_uv_re(ft):
                fsl = slice(ft * 4, (ft + 1) * 4)
                u1 = uvpool.tile([128, BPC, 64], f32, tag="u1", name=f"u1_{ft}")
                v1 = uvpool.tile([128, BPC, 64], f32, tag="v1", name=f"v1_{ft}")
                nc.gpsimd.tensor_tensor(u1[:], _bcast(cphi_sb, ft),
                                        _sbcast(sre_sb, fsl), op=mult)
                nc.gpsimd.tensor_tensor(v1[:], _bcast(sphi_sb, ft),
                                        _sbcast(sre_sb, fsl), op=mult)
                return u1, v1

            def emit_uv_im(ft, u1, v1):
                # for the final tile the GpSimd chain is the serial tail;
                # the DVE is idle by then and ~1.7x faster per op
                eng = nc.vector if ft == FT - 2 else nc.gpsimd
                fsl = slice(ft * 4, (ft + 1) * 4)
                u2 = uvpool.tile([128, BPC, 64], f32, tag="u2", name=f"u2_{ft}")
                v2 = uvpool.tile([128, BPC, 64], f32, tag="v2", name=f"v2_{ft}")
                Ut = uvpool.tile([128, BPC, 64], bf16, tag="U", name=f"U_{ft}")
                Vt = uvpool.tile([128, BPC, 64], bf16, tag="V", name=f"V_{ft}")
                eng.tensor_tensor(u2[:], _bcast(sphi_sb, ft),
                                  _sbcast(sim_sb, fsl), op=mult)
                eng.tensor_tensor(v2[:], _bcast(cphi_sb, ft),
                                  _sbcast(sim_sb, fsl), op=mult)
                eng.tensor_tensor(Ut[:], u1[:], u2[:], op=subtract)
                eng.tensor_tensor(Vt[:], v1[:], v2[:], op=add)
                pend.append((ft, (Ut, Vt)))

            def emit_uv46():
                # k=4096 bin: S_im == 0 there, so U = cphi*S_re, V = sphi*S_re
                ft = FT - 1
                fsl = slice(ft * 4, (ft + 1) * 4)
                Ut = uvpool.tile([128, BPC, 64], bf16, tag="U", name="U_46")
                Vt = uvpool.tile([128, BPC, 64], bf16, tag="V", name="V_46")
                nc.gpsimd.tensor_tensor(Ut[:], _bcast(cphi_sb, ft),
                                        _sbcast(sre_sb, fsl), op=mult)
                nc.gpsimd.tensor_tensor(Vt[:], _bcast(sphi_sb, ft),
                                        _sbcast(sre_sb, fsl), op=mult)
                pend.append((ft, (Ut, Vt)))

            # k=4096 bin tile (ft == FT-1) runs first: it needs only x.
            for ft in [FT - 1] + list(range(FT - 1)):
                if ft == FT - 1:
                    # k=4096 bin: P1/P2 via 1-column matmuls onto partition 0.
                    # cphi row 4096 is (-1)^s/P (sphi row 0, cosa row 0 all-1,
                    # nsina row 0 all-0), so the generic twiddle+IFFT path
                    # below handles this bin exactly.
                    ps46 = {}
                    for hi, c0 in ((0, 0), (1, HT)):
                        for col in (0, 1):
                            ps = pstpool.tile([128, HT], f32, tag="pst",
                                              name=f"ps46_{col}_{hi}")
                            for ck in range(4):
                                nc.tensor.matmul(
                                    ps[0:1, :],
                                    a46_sb[:, ck, col:col + 1],
                                    x_sb[:, ck, c0:c0 + HT],
                                    start=(ck == 0),
                                    stop=(ck == 3),
                                )
                            ps46[(col, hi)] = ps
                    c46 = {}
                    for hi in (0, 1):
                        c_t = scr.tile([128, HT], f32, tag=f"c2{hi}",
                                       name=f"c46_{hi}")
                        nc.scalar.activation(c_t[0:1, :], ps46[(1, hi)][0:1, :],
                                             Act.Copy)
                        c46[hi] = c_t
                    for bl in range(BPC):
                        idx = ft * 4 + bl
                        hi, s0 = bl // 2, (bl % 2) * HW
                        seg = slice(s0, s0 + HW)
                        sc = scr.tile([128, HW], f32, tag="sc0",
                                      name=f"sc46_{bl}")
                        nc.vector.scalar_tensor_tensor(
                            sc[0:1, :], ps46[(0, hi)][0:1, seg], 1.0,
                            c46[hi][0:1, seg],
                            bypass, mult,
                            accum_out=sre_sb[0:1, idx:idx + 1])
                else:
                    a_t = apool.tile([128, 4, 512], bf16, tag="a")
                    nc.sync.dma_start(a_t[:], a_d[ft])
                if ft == 4:
                    # bulky twiddle constants: sync-queue position after a4
                    # so x/a0..a4 get the full HBM bandwidth first (GpSimd
                    # only needs them ~4 periods in).  The scheduler keeps
                    # same-queue DMA emission order; the Act queue does not.
                    # All uv emission for earlier tiles is deferred to here:
                    # a cphi/sphi read emitted before these dma_starts would
                    # order the DMA after the read and compute on garbage.
                    nc.sync.dma_start(cphi_sb[:], cphi_d)
                    nc.sync.dma_start(sphi_sb[:], sphi_d)
                    nc.sync.dma_start(cosa_sb[:], cosa_d)
                    nc.sync.dma_start(nsina_sb[:], nsina_d)
                    emit_uv46()
                    for f in range(4):
                        emit_uv_im(f, *emit_uv_re(f))
                psm = {}
                cpy = {}
                # m=0 first so ps0 exists when c2's copy lands and the DVE
                # chain starts ~1.4us earlier; components 2,3 get Act
                # evacuation to SBUF (the stt in1 operand); 0,1 stay in
                # PSUM so each stt uses only one SBUF read port
                for m in (0, 2, 3, 1) if ft < FT - 1 else ():
                    msl = slice(m * 128, (m + 1) * 128)
                    for hi, c0 in ((0, 0), (1, HT)):
                        ps = pstpool.tile([128, HT], f32, tag="pst",
                                          name=f"ps{m}_{hi}_{ft}")
                        for ck in range(4):
                            nc.tensor.matmul(
                                ps[:, :],
                                a_t[:, ck, msl],
                                x_sb[:, ck, c0:c0 + HT],
                                start=(ck == 0),
                                stop=(ck == 3),
                            )
                        psm[(m, hi)] = ps
                        if m in (2, 3):
                            c_m = scr.tile([128, HT], f32, tag=f"c{m}{hi}",
                                           name=f"c{m}{hi}_{ft}")
                            nc.scalar.activation(c_m[:], ps[:], Act.Copy)
                            cpy[(m, hi)] = c_m

                # A=sum p0*p2, B=sum p1*p3, C=sum p0*p3, D=sum p1*p2.
                # A,B first so S_re = A-B is ready mid-chain and GpSimd's
                # re-products overlap the C,D stts.
                def stt_chain(tg, pa, cb, dst):
                    for bl in range(BPC):
                        idx = ft * 4 + bl
                        hi, s0 = bl // 2, (bl % 2) * HW
                        seg = slice(s0, s0 + HW)
                        sc = scr.tile([128, HW], f32, tag=tg,
                                      name=f"{tg}_{ft}_{bl}")
                        nc.vector.scalar_tensor_tensor(
                            sc[:], psm[(pa, hi)][:, seg], 1.0,
                            cpy[(cb, hi)][:, seg],
                            bypass, mult,
                            accum_out=dst[:, idx:idx + 1])

                # chain order A, C, B, D matches input arrival (ps0 and the
                # c2/c3 copies come before ps1) so the DVE never idles
                fsl = slice(ft * 4, (ft + 1) * 4)
                if ft < FT - 1:
                    stt_chain("sc0", 0, 2, sA_sb)
                    stt_chain("sc2", 0, 3, sC_sb)
                    stt_chain("sc1", 1, 3, sB_sb)
                    nc.vector.tensor_sub(sre_sb[:, fsl], sA_sb[:, fsl],
                                         sB_sb[:, fsl])
                    u1 = v1 = None
                    if ft >= 4:
                        u1, v1 = emit_uv_re(ft)
                    stt_chain("sc3", 1, 2, sD_sb)
                    nc.vector.tensor_add(sim_sb[:, fsl], sC_sb[:, fsl],
                                         sD_sb[:, fsl])
                    if ft >= 4:
                        emit_uv_im(ft, u1, v1)

                # emit IFFT matmuls 3 iterations late to keep the PE queue fed
                while len(pend) > 3:
                    emit_stage_c(*pend.pop(0))
            while pend:
                emit_stage_c(*pend.pop(0))

            # ---- stage D: signed sqrt, per-batch l2 norm, store ----
            # bf16 |Y| feeds the norm matmul (1-pass bf16 instead of the
            # double-pass fp32 LOW_HIGH) and the Sqrt; error ~0.2% final
            absy = const.tile([128, BPC * 64], bf16)
            nc.scalar.activation(absy[:], psy[:], Act.Abs)
            sqy = const.tile([128, BPC * 64], f32)
            nc.scalar.activation(sqy[:], absy[:], Act.Sqrt)
            sgn = const.tile([128, BPC * 64], f32)
            nc.scalar.activation(sgn[:], psy[:], Act.Sign)
            ys = const.tile([128, BPC * 64], f32)
            nc.vector.tensor_mul(ys[:], sqy[:], sgn[:])

            # norm^2 per batch = sum_p y^2 = sum_p |Y|  (Y = pre-sqrt value).
            # All-ones [128,128] stationary replicates the column sums to
            # every partition, so no extra broadcast matmul is needed for
            # the final scale.
            psn = pstpool.tile([128, BPC * 64], f32, tag="pst", name="psn")
            nc.tensor.matmul(psn[:, :], ones_sb[:], absy[:],
                             start=True, stop=True)
            nsq = const.tile([128, BPC], f32)
            nc.vector.reduce_sum(
                out=nsq[:],
                in_=psn[:, :].rearrange("p (b s) -> p b s", b=BPC),
                axis=mybir.AxisListType.X,
            )
            nc.vector.tensor_scalar_max(nsq[:], nsq[:], 1e-10)
            sqn = const.tile([128, BPC], f32)
            nc.scalar.activation(sqn[:], nsq[:], Act.Sqrt)
            invn = const.tile([128, BPC], f32)
            nc.vector.reciprocal(invn[:], sqn[:])

            inv_b = invn[:, :, None].broadcast_to([128, BPC, 64])
            fin = const.tile([128, BPC * 64], f32)
            # two halves: the first store's transfer overlaps the second
            # half's scale + descriptor write
            for h0 in (0, 2):
                bs = slice(h0 * 64, (h0 + 2) * 64)
                nc.vector.tensor_tensor(
                    fin[:, bs].rearrange("p (b s) -> p b s", b=2),
                    ys[:, bs].rearrange("p (b s) -> p b s", b=2),
                    inv_b[:, h0:h0 + 2, :],
                    op=mult,
                )
                nc.sync.dma_start(
                    y_d[h0:h0 + 2].rearrange("b (q s) -> q b s", q=128),
                    fin[:, bs].rearrange("p (b s) -> p b s", b=2),
                )

    nc.compile()
    return nc


def _to_bf16(a):
    import ml_dtypes
    return np.asarray(a, np.float32).astype(ml_dtypes.bfloat16)


def _host_prep(x, M1, M2):
    x = np.ascontiguousarray(np.asarray(x, np.float32))
    M1 = np.asarray(M1, np.float32)
    M2 = np.asarray(M2, np.float32)

    h1 = np.argmax(np.abs(M1), axis=1)
    s1 = M1[np.arange(C), h1].astype(np.float64)
    h2 = np.argmax(np.abs(M2), axis=1)
    s2 = M2[np.arange(C), h2].astype(np.float64)

    k = np.arange(NSLOT, dtype=np.float64)
    valid = k <= P // 2
    ang1 = 2 * np.pi * np.outer(h1.astype(np.float64), k) / P
    ang2 = 2 * np.pi * np.outer(h2.astype(np.float64), k) / P
    # a[ft, c, m*128 + j]: m in (A1re, A1im, A2re, A2im), freq = ft*128 + j
    a = np.empty((FT - 1, C, 512), np.float32)
    a1re = (s1[:, None] * np.cos(ang1) * valid).astype(np.float32)
    a1im = (-s1[:, None] * np.sin(ang1) * valid).astype(np.float32)
    a2re = (s2[:, None] * np.cos(ang2) * valid).astype(np.float32)
    a2im = (-s2[:, None] * np.sin(ang2) * valid).astype(np.float32)
    for ft in range(FT - 1):
        ksl = slice(ft * 128, (ft + 1) * 128)
        a[ft, :, 0:128] = a1re[:, ksl]
        a[ft, :, 128:256] = a1im[:, ksl]
        a[ft, :, 256:384] = a2re[:, ksl]
        a[ft, :, 384:512] = a2im[:, ksl]

    w = np.where(valid, 2.0 / P, 0.0)
    w[0] = 1.0 / P
    w[P // 2] = 1.0 / P
    s_idx = np.arange(64, dtype=np.float64)
    phi = 2 * np.pi * np.outer(k, s_idx) / P
    cphi = (w[:, None] * np.cos(phi)).astype(np.float32).reshape(FT, 128, 64)
    sphi = (w[:, None] * np.sin(phi)).astype(np.float32).reshape(FT, 128, 64)

    km = np.arange(128, dtype=np.float64)
    alpha = 2 * np.pi * np.outer(km, km) / 128
    cosa = np.cos(alpha).astype(np.float32)
    nsina = (-np.sin(alpha)).astype(np.float32)

    xt = np.ascontiguousarray(x.reshape(B * HW, C).T)  # [C, 6272]

    # pre-transposed, DMA-contiguous layouts
    a_t = np.ascontiguousarray(
        a.reshape(FT - 1, 4, 128, 512).transpose(0, 2, 1, 3))
    # k=4096 columns: a46[p, ck, 0/1] = s1/2[c] * cos(pi*h1/2[c]), c=ck*128+p
    a46 = np.stack([a1re[:, 4096], a2re[:, 4096]], axis=1)  # [C, 2]
    a46 = np.ascontiguousarray(a46.reshape(4, 128, 2).transpose(1, 0, 2))
    cphi_t = np.ascontiguousarray(cphi.transpose(1, 0, 2))  # [128p,FT,64]
    sphi_t = np.ascontiguousarray(sphi.transpose(1, 0, 2))
    xt_t = xt.reshape(4, 128, B * HW).transpose(1, 0, 2)    # [128p,4ck,T]
    return (_to_bf16(a_t), _to_bf16(a46), cphi_t, sphi_t, _to_bf16(cosa),
            _to_bf16(nsina), _to_bf16(xt_t))


def _make_in_maps(x, M1, M2):
    a, a46, cphi, sphi, cosa, nsina, xt = _host_prep(x, M1, M2)
    in_maps = []
    for r in range(NCORES):
        in_maps.append({
            "a": a,
            "a46": a46,
            "x": np.ascontiguousarray(xt[:, :, r * T:(r + 1) * T]),
            "cphi": cphi,
            "sphi": sphi,
            "cosa": cosa,
            "nsina": nsina,
        })
    return in_maps


def kernel(x, M1, M2):
    from concourse.bass_utils import run_bass_kernel_spmd

    if "nc" not in _CACHE:
        _CACHE["nc"] = _build_program()
    nc = _CACHE["nc"]

    in_maps = _make_in_maps(x, M1, M2)
    res = run_bass_kernel_spmd(nc, in_maps, core_ids=list(range(NCORES)))
    out = np.concatenate([res.results[r]["y"] for r in range(NCORES)], axis=0)
    return out.astype(np.float32)


# revision 45
# speedup vs baseline: 1.3065x; 1.0040x over previous
"""Trainium2 kernel for CompactBilinearLayer (count-sketch bilinear pooling).

Math: reference computes y = l2norm(signed_sqrt(sum_hw Re IFFT(FFT(x@M1)*FFT(x@M2)))).
Since M1/M2 are count-sketch matrices (one +-1 per row), FFT(x@M1) == x @ A1 with
A1[c,k] = s1[c] * exp(-2pi i h1[c] k / P) — a dense [512, K] matrix computable on the
host from M1 in O(C*K). The IFFT is linear, so the spatial sum moves before it.
Hermitian symmetry means only k = 0..4096 are needed.  Per core (4 batch elements,
784 spatial positions — fully batch-local, no collectives):
  A: P1/P2 projections = A^T @ x^T, single-pass bf16 matmuls; per-component
     pairs of 1-bank [128,392] PSUM tiles (7-deep ring) so Act evacuation
     copies start right after each half-group and never gate the next tile
  B: S[k,b] = sum_t (P1*P2) per batch via fused DVE scalar_tensor_tensor
     (product+reduce in one op), one PSUM + one SBUF operand each.  A,B
     chains run before C,D so S_re is ready mid-chain and GpSimd's
     re-products overlap the remaining stts.
  C: IFFT via two-step factorization n=64q+s: GpSimd computes the twiddle
     products and folds them to U = u1-u2, V = v1+v2 (bf16), PE accumulates
     just 2 bf16 matmuls over k%128 into psy.  Stage-C matmuls are emitted
     3 iterations late so the in-order PE queue never waits on the
     DVE->GpSimd chain; the final tile's fold runs on the (by then idle) DVE.
  D: signed sqrt + per-batch L2 norm (all-partition replicated column sums,
     no broadcast matmul) + two half stores so the first transfer overlaps
     the second half's scale.
The k=4096 bin tile runs FIRST (it only needs x, not the streamed A tiles),
overlapping the startup DMA window.  The 13-deep A-tile pool gives the
single Sync DGE queue enough issue lead; the bulky twiddle constants are
queued behind a0..a4 so x and the first A tiles get full HBM bandwidth
(their consumers, uv products for tiles 0..3, are deferred past those
dma_starts — a cphi read emitted earlier would order the DMA after it).
"""
import numpy as np

P = 8192
C = 512
FT = 33            # frequency tiles of 128 -> 4224 slots >= 4097
NSLOT = FT * 128
NCORES = 8
BPC = 4            # batch elems per core
HW = 196           # spatial positions per batch elem
T = BPC * HW       # 784 positions per core
HT = T // 2        # 392, one PSUM bank of f32
B = 32

_CACHE = {}


def _build_program():
    import concourse.bass as bass
    import concourse.tile as tile
    from concourse import bacc, mybir

    f32 = mybir.dt.float32
    f32r = mybir.dt.float32r
    bf16 = mybir.dt.bfloat16
    nc = bacc.Bacc("TRN2", target_bir_lowering=False, debug=False,
                   num_devices=NCORES)

    a_d = nc.dram_tensor("a", [FT - 1, 128, 4, 512], bf16, kind="ExternalInput").ap()
    a46_d = nc.dram_tensor("a46", [128, 4, 2], bf16, kind="ExternalInput").ap()
    x_d = nc.dram_tensor("x", [128, 4, T], bf16, kind="ExternalInput").ap()
    cphi_d = nc.dram_tensor("cphi", [128, FT, 64], f32, kind="ExternalInput").ap()
    sphi_d = nc.dram_tensor("sphi", [128, FT, 64], f32, kind="ExternalInput").ap()
    cosa_d = nc.dram_tensor("cosa", [128, 128], bf16, kind="ExternalInput").ap()
    ncosa_d = nc.dram_tensor("ncosa", [128, 128], bf16, kind="ExternalInput").ap()
    nsina_d = nc.dram_tensor("nsina", [128, 128], bf16, kind="ExternalInput").ap()
    y_d = nc.dram_tensor("y", [BPC, P], f32, kind="ExternalOutput").ap()

    mult = mybir.AluOpType.mult
    subtract = mybir.AluOpType.subtract
    add = mybir.AluOpType.add
    bypass = mybir.AluOpType.bypass
    Act = mybir.ActivationFunctionType

    with tile.TileContext(nc) as tc:
        with (
            tc.tile_pool(name="const", bufs=1) as const,
            tc.tile_pool(name="apool", bufs=13) as apool,
            tc.tile_pool(name="pst", bufs=7, space="PSUM") as pstpool,
            tc.tile_pool(name="psyp", bufs=1, space="PSUM") as psypool,
            tc.tile_pool(name="scr", bufs=4) as scr,
            tc.tile_pool(name="uv", bufs=7) as uvpool,
        ):
            a46_sb = const.tile([128, 4, 2], bf16)
            nc.sync.dma_start(a46_sb[:], a46_d)
            x_sb = const.tile([128, 4, T], bf16)
            # two descriptors: the first ft46 matmuls only touch t < HT,
            # so they can start ~1us before the second half lands
            nc.sync.dma_start(x_sb[:, :, 0:HT], x_d[:, :, 0:HT])
            nc.sync.dma_start(x_sb[:, :, HT:T], x_d[:, :, HT:T])
            cphi_sb = const.tile([128, FT, 64], f32)
            sphi_sb = const.tile([128, FT, 64], f32)
            cosa_sb = const.tile([128, 128], bf16)
            ncosa_sb = const.tile([128, 128], bf16)
            nsina_sb = const.tile([128, 128], bf16)
            ones_sb = const.tile([128, 128], bf16)
            nc.vector.memset(ones_sb[:], 1.0)
            # preload the Abs/Sqrt/Sign activation tables during the initial
            # DMA window so stage D doesn't pay the ~2.6us table switch
            warm = const.tile([1, 1], f32)
            nc.vector.memset(warm[:], 1.0)
            wo = const.tile([1, 1], f32)
            nc.scalar.activation(wo[:], warm[:], Act.Abs)
            nc.scalar.activation(wo[:], wo[:], Act.Sqrt)
            nc.scalar.activation(wo[:], wo[:], Act.Sign)
            sre_sb = const.tile([128, FT * 4], f32)
            sim_sb = const.tile([128, FT * 4], f32)
            nc.vector.memset(sre_sb[:, (FT - 1) * 4:], 0.0)
            nc.vector.memset(sim_sb[:, (FT - 1) * 4:], 0.0)
            sA_sb = const.tile([128, FT * 4], f32)
            sB_sb = const.tile([128, FT * 4], f32)
            sC_sb = const.tile([128, FT * 4], f32)
            sD_sb = const.tile([128, FT * 4], f32)

            psy = psypool.tile([128, BPC * 64], f32, tag="psy")

            # the 3 flush tiles use the unfolded 4-matmul form: their extra
            # PE work lands in the PE-idle tail while GpSimd/DVE skip the
            # U/V combine ops on the serial drain path
            N4 = 3
            FT4 = set(range(FT - 1 - N4, FT - 1))
            n_emitted = [0]
            N_CMM = 2 * (FT - N4) + 4 * N4  # total stage-C matmuls

            def emit_stage_c(ft, us):
                if len(us) == 2:
                    pairs = ((cosa_sb, us[0]), (nsina_sb, us[1]))
                else:
                    u1, u2, v1, v2 = us
                    pairs = ((cosa_sb, u1), (ncosa_sb, u2),
                             (nsina_sb, v1), (nsina_sb, v2))
                for w, rhs in pairs:
                    nc.tensor.matmul(psy[:], w[:],
                                     rhs[:].rearrange("p b s -> p (b s)"),
                                     start=(n_emitted[0] == 0),
                                     stop=(n_emitted[0] == N_CMM - 1))
                    n_emitted[0] += 1

            pend = []

            def _bcast(src, ft):
                return src[:, ft, :][:, None, :].broadcast_to([128, BPC, 64])

            def _sbcast(src, fsl):
                return src[:, fsl][:, :, None].broadcast_to([128, BPC, 64])

            # twiddle products on GpSimd (u = phi * S, broadcast both ways),
            # folded to U = u1-u2, V = v1+v2 so the PE only needs 2 IFFT
            # matmuls per tile.  Split in two halves: the re-products depend
            # only on S_re and are emitted mid-way through the DVE chain so
            # GpSimd overlaps the remaining stts (shortens the serial tail).
            def emit_uv_re(ft):
                fsl = slice(ft * 4, (ft + 1) * 4)
                dt = bf16 if ft in FT4 else f32
                u1 = uvpool.tile([128, BPC, 64], dt, tag="u1", name=f"u1_{ft}")
                v1 = uvpool.tile([128, BPC, 64], dt, tag="v1", name=f"v1_{ft}")
                nc.gpsimd.tensor_tensor(u1[:], _bcast(cphi_sb, ft),
                                        _sbcast(sre_sb, fsl), op=mult)
                nc.gpsimd.tensor_tensor(v1[:], _bcast(sphi_sb, ft),
                                        _sbcast(sre_sb, fsl), op=mult)
                return u1, v1

            def emit_uv_im(ft, u1, v1):
                # for the final tile the GpSimd chain is the serial tail;
                # the DVE is idle by then and ~1.7x faster per op
                eng = nc.vector if ft == FT - 2 else nc.gpsimd
                fsl = slice(ft * 4, (ft + 1) * 4)
                dt = bf16 if ft in FT4 else f32
                u2 = uvpool.tile([128, BPC, 64], dt, tag="u2", name=f"u2_{ft}")
                v2 = uvpool.tile([128, BPC, 64], dt, tag="v2", name=f"v2_{ft}")
                eng.tensor_tensor(u2[:], _bcast(sphi_sb, ft),
                                  _sbcast(sim_sb, fsl), op=mult)
                eng.tensor_tensor(v2[:], _bcast(cphi_sb, ft),
                                  _sbcast(sim_sb, fsl), op=mult)
                if ft in FT4:
                    pend.append((ft, (u1, u2, v1, v2)))
                    return
                Ut = uvpool.tile([128, BPC, 64], bf16, tag="U", name=f"U_{ft}")
                Vt = uvpool.tile([128, BPC, 64], bf16, tag="V", name=f"V_{ft}")
                eng.tensor_tensor(Ut[:], u1[:], u2[:], op=subtract)
                eng.tensor_tensor(Vt[:], v1[:], v2[:], op=add)
                pend.append((ft, (Ut, Vt)))

            def emit_uv46():
                # k=4096 bin: S_im == 0 there, so U = cphi*S_re, V = sphi*S_re
                ft = FT - 1
                fsl = slice(ft * 4, (ft + 1) * 4)
                Ut = uvpool.tile([128, BPC, 64], bf16, tag="U", name="U_46")
                Vt = uvpool.tile([128, BPC, 64], bf16, tag="V", name="V_46")
                nc.gpsimd.tensor_tensor(Ut[:], _bcast(cphi_sb, ft),
                                        _sbcast(sre_sb, fsl), op=mult)
                nc.gpsimd.tensor_tensor(Vt[:], _bcast(sphi_sb, ft),
                                        _sbcast(sre_sb, fsl), op=mult)
                pend.append((ft, (Ut, Vt)))

            # k=4096 bin tile (ft == FT-1) runs first: it needs only x.
            for ft in [FT - 1] + list(range(FT - 1)):
                if ft == FT - 1:
                    # k=4096 bin: P1/P2 via 1-column matmuls onto partition 0.
                    # cphi row 4096 is (-1)^s/P (sphi row 0, cosa row 0 all-1,
                    # nsina row 0 all-0), so the generic twiddle+IFFT path
                    # below handles this bin exactly.
                    ps46 = {}
                    for hi, c0 in ((0, 0), (1, HT)):
                        for col in (0, 1):
                            ps = pstpool.tile([128, HT], f32, tag="pst",
                                              name=f"ps46_{col}_{hi}")
                            for ck in range(4):
                                nc.tensor.matmul(
                                    ps[0:1, :],
                                    a46_sb[:, ck, col:col + 1],
                                    x_sb[:, ck, c0:c0 + HT],
                                    start=(ck == 0),
                                    stop=(ck == 3),
                                )
                            ps46[(col, hi)] = ps
                    c46 = {}
                    for hi in (0, 1):
                        c_t = scr.tile([128, HT], f32, tag=f"c2{hi}",
                                       name=f"c46_{hi}")
                        nc.scalar.activation(c_t[0:1, :], ps46[(1, hi)][0:1, :],
                                             Act.Copy)
                        c46[hi] = c_t
                    for bl in range(BPC):
                        idx = ft * 4 + bl
                        hi, s0 = bl // 2, (bl % 2) * HW
                        seg = slice(s0, s0 + HW)
                        sc = scr.tile([128, HW], f32, tag="sc0",
                                      name=f"sc46_{bl}")
                        nc.vector.scalar_tensor_tensor(
                            sc[0:1, :], ps46[(0, hi)][0:1, seg], 1.0,
                            c46[hi][0:1, seg],
                            bypass, mult,
                            accum_out=sre_sb[0:1, idx:idx + 1])
                else:
                    a_t = apool.tile([128, 4, 512], bf16, tag="a")
                    nc.sync.dma_start(a_t[:], a_d[ft])
                if ft == 4:
                    # bulky twiddle constants: sync-queue position after a4
                    # so x/a0..a4 get the full HBM bandwidth first (GpSimd
                    # only needs them ~4 periods in).  The scheduler keeps
                    # same-queue DMA emission order; the Act queue does not.
                    # All uv emission for earlier tiles is deferred to here:
                    # a cphi/sphi read emitted before these dma_starts would
                    # order the DMA after the read and compute on garbage.
                    nc.sync.dma_start(cphi_sb[:], cphi_d)
                    nc.sync.dma_start(sphi_sb[:], sphi_d)
                    nc.sync.dma_start(cosa_sb[:], cosa_d)
                    nc.sync.dma_start(ncosa_sb[:], ncosa_d)
                    nc.sync.dma_start(nsina_sb[:], nsina_d)
                    emit_uv46()
                    for f in range(4):
                        emit_uv_im(f, *emit_uv_re(f))
                psm = {}
                cpy = {}
                # m=0 first so ps0 exists when c2's copy lands and the DVE
                # chain starts ~1.4us earlier; components 2,3 get Act
                # evacuation to SBUF (the stt in1 operand); 0,1 stay in
                # PSUM so each stt uses only one SBUF read port
                for m in (0, 2, 3, 1) if ft < FT - 1 else ():
                    msl = slice(m * 128, (m + 1) * 128)
                    for hi, c0 in ((0, 0), (1, HT)):
                        ps = pstpool.tile([128, HT], f32, tag="pst",
                                          name=f"ps{m}_{hi}_{ft}")
                        for ck in range(4):
                            nc.tensor.matmul(
                                ps[:, :],
                                a_t[:, ck, msl],
                                x_sb[:, ck, c0:c0 + HT],
                                start=(ck == 0),
                                stop=(ck == 3),
                            )
                        psm[(m, hi)] = ps
                        if m in (2, 3):
                            c_m = scr.tile([128, HT], f32, tag=f"c{m}{hi}",
                                           name=f"c{m}{hi}_{ft}")
                            nc.scalar.activation(c_m[:], ps[:], Act.Copy)
                            cpy[(m, hi)] = c_m

                # A=sum p0*p2, B=sum p1*p3, C=sum p0*p3, D=sum p1*p2.
                # A,B first so S_re = A-B is ready mid-chain and GpSimd's
                # re-products overlap the C,D stts.
                def stt_chain(tg, pa, cb, dst):
                    for bl in range(BPC):
                        idx = ft * 4 + bl
                        hi, s0 = bl // 2, (bl % 2) * HW
                        seg = slice(s0, s0 + HW)
                        sc = scr.tile([128, HW], f32, tag=tg,
                                      name=f"{tg}_{ft}_{bl}")
                        nc.vector.scalar_tensor_tensor(
                            sc[:], psm[(pa, hi)][:, seg], 1.0,
                            cpy[(cb, hi)][:, seg],
                            bypass, mult,
                            accum_out=dst[:, idx:idx + 1])

                # chain order A, C, B, D matches input arrival (ps0 and the
                # c2/c3 copies come before ps1) so the DVE never idles
                fsl = slice(ft * 4, (ft + 1) * 4)
                if ft < FT - 1:
                    stt_chain("sc0", 0, 2, sA_sb)
                    stt_chain("sc2", 0, 3, sC_sb)
                    stt_chain("sc1", 1, 3, sB_sb)
                    nc.vector.tensor_sub(sre_sb[:, fsl], sA_sb[:, fsl],
                                         sB_sb[:, fsl])
                    u1 = v1 = None
                    if ft >= 4:
                        u1, v1 = emit_uv_re(ft)
                    stt_chain("sc3", 1, 2, sD_sb)
                    nc.vector.tensor_add(sim_sb[:, fsl], sC_sb[:, fsl],
                                         sD_sb[:, fsl])
                    if ft >= 4:
                        emit_uv_im(ft, u1, v1)

                # emit IFFT matmuls 3 iterations late to keep the PE queue fed
                while len(pend) > 3:
                    emit_stage_c(*pend.pop(0))
            while pend:
                emit_stage_c(*pend.pop(0))

            # ---- stage D: signed sqrt, per-batch l2 norm, store ----
            # bf16 |Y| feeds the norm matmul (1-pass bf16 instead of the
            # double-pass fp32 LOW_HIGH) and the Sqrt; error ~0.2% final
            absy = const.tile([128, BPC * 64], bf16)
            nc.scalar.activation(absy[:], psy[:], Act.Abs)
            sqy = const.tile([128, BPC * 64], f32)
            nc.scalar.activation(sqy[:], absy[:], Act.Sqrt)
            sgn = const.tile([128, BPC * 64], f32)
            nc.scalar.activation(sgn[:], psy[:], Act.Sign)
            ys = const.tile([128, BPC * 64], f32)
            nc.vector.tensor_mul(ys[:], sqy[:], sgn[:])

            # norm^2 per batch = sum_p y^2 = sum_p |Y|  (Y = pre-sqrt value).
            # All-ones [128,128] stationary replicates the column sums to
            # every partition, so no extra broadcast matmul is needed for
            # the final scale.
            psn = pstpool.tile([128, BPC * 64], f32, tag="pst", name="psn")
            nc.tensor.matmul(psn[:, :], ones_sb[:], absy[:],
                             start=True, stop=True)
            nsq = const.tile([128, BPC], f32)
            nc.vector.reduce_sum(
                out=nsq[:],
                in_=psn[:, :].rearrange("p (b s) -> p b s", b=BPC),
                axis=mybir.AxisListType.X,
            )
            nc.vector.tensor_scalar_max(nsq[:], nsq[:], 1e-10)
            sqn = const.tile([128, BPC], f32)
            nc.scalar.activation(sqn[:], nsq[:], Act.Sqrt)
            invn = const.tile([128, BPC], f32)
            nc.vector.reciprocal(invn[:], sqn[:])

            inv_b = invn[:, :, None].broadcast_to([128, BPC, 64])
            fin = const.tile([128, BPC * 64], f32)
            # two halves: the first store's transfer overlaps the second
            # half's scale + descriptor write
            for h0 in (0, 2):
                bs = slice(h0 * 64, (h0 + 2) * 64)
                nc.vector.tensor_tensor(
                    fin[:, bs].rearrange("p (b s) -> p b s", b=2),
                    ys[:, bs].rearrange("p (b s) -> p b s", b=2),
                    inv_b[:, h0:h0 + 2, :],
                    op=mult,
                )
                nc.sync.dma_start(
                    y_d[h0:h0 + 2].rearrange("b (q s) -> q b s", q=128),
                    fin[:, bs].rearrange("p (b s) -> p b s", b=2),
                )

    nc.compile()
    return nc


def _to_bf16(a):
    import ml_dtypes
    return np.asarray(a, np.float32).astype(ml_dtypes.bfloat16)


def _host_prep(x, M1, M2):
    x = np.ascontiguousarray(np.asarray(x, np.float32))
    M1 = np.asarray(M1, np.float32)
    M2 = np.asarray(M2, np.float32)

    h1 = np.argmax(np.abs(M1), axis=1)
    s1 = M1[np.arange(C), h1].astype(np.float64)
    h2 = np.argmax(np.abs(M2), axis=1)
    s2 = M2[np.arange(C), h2].astype(np.float64)

    k = np.arange(NSLOT, dtype=np.float64)
    valid = k <= P // 2
    ang1 = 2 * np.pi * np.outer(h1.astype(np.float64), k) / P
    ang2 = 2 * np.pi * np.outer(h2.astype(np.float64), k) / P
    # a[ft, c, m*128 + j]: m in (A1re, A1im, A2re, A2im), freq = ft*128 + j
    a = np.empty((FT - 1, C, 512), np.float32)
    a1re = (s1[:, None] * np.cos(ang1) * valid).astype(np.float32)
    a1im = (-s1[:, None] * np.sin(ang1) * valid).astype(np.float32)
    a2re = (s2[:, None] * np.cos(ang2) * valid).astype(np.float32)
    a2im = (-s2[:, None] * np.sin(ang2) * valid).astype(np.float32)
    for ft in range(FT - 1):
        ksl = slice(ft * 128, (ft + 1) * 128)
        a[ft, :, 0:128] = a1re[:, ksl]
        a[ft, :, 128:256] = a1im[:, ksl]
        a[ft, :, 256:384] = a2re[:, ksl]
        a[ft, :, 384:512] = a2im[:, ksl]

    w = np.where(valid, 2.0 / P, 0.0)
    w[0] = 1.0 / P
    w[P // 2] = 1.0 / P
    s_idx = np.arange(64, dtype=np.float64)
    phi = 2 * np.pi * np.outer(k, s_idx) / P
    cphi = (w[:, None] * np.cos(phi)).astype(np.float32).reshape(FT, 128, 64)
    sphi = (w[:, None] * np.sin(phi)).astype(np.float32).reshape(FT, 128, 64)

    km = np.arange(128, dtype=np.float64)
    alpha = 2 * np.pi * np.outer(km, km) / 128
    cosa = np.cos(alpha).astype(np.float32)
    nsina = (-np.sin(alpha)).astype(np.float32)

    xt = np.ascontiguousarray(x.reshape(B * HW, C).T)  # [C, 6272]

    # pre-transposed, DMA-contiguous layouts
    a_t = np.ascontiguousarray(
        a.reshape(FT - 1, 4, 128, 512).transpose(0, 2, 1, 3))
    # k=4096 columns: a46[p, ck, 0/1] = s1/2[c] * cos(pi*h1/2[c]), c=ck*128+p
    a46 = np.stack([a1re[:, 4096], a2re[:, 4096]], axis=1)  # [C, 2]
    a46 = np.ascontiguousarray(a46.reshape(4, 128, 2).transpose(1, 0, 2))
    cphi_t = np.ascontiguousarray(cphi.transpose(1, 0, 2))  # [128p,FT,64]
    sphi_t = np.ascontiguousarray(sphi.transpose(1, 0, 2))
    xt_t = xt.reshape(4, 128, B * HW).transpose(1, 0, 2)    # [128p,4ck,T]
    return (_to_bf16(a_t), _to_bf16(a46), cphi_t, sphi_t, _to_bf16(cosa),
            _to_bf16(-cosa), _to_bf16(nsina), _to_bf16(xt_t))


def _make_in_maps(x, M1, M2):
    a, a46, cphi, sphi, cosa, ncosa, nsina, xt = _host_prep(x, M1, M2)
    in_maps = []
    for r in range(NCORES):
        in_maps.append({
            "a": a,
            "a46": a46,
            "x": np.ascontiguousarray(xt[:, :, r * T:(r + 1) * T]),
            "cphi": cphi,
            "sphi": sphi,
            "cosa": cosa,
            "ncosa": ncosa,
            "nsina": nsina,
        })
    return in_maps


def kernel(x, M1, M2):
    from concourse.bass_utils import run_bass_kernel_spmd

    if "nc" not in _CACHE:
        _CACHE["nc"] = _build_program()
    nc = _CACHE["nc"]

    in_maps = _make_in_maps(x, M1, M2)
    res = run_bass_kernel_spmd(nc, in_maps, core_ids=list(range(NCORES)))
    out = np.concatenate([res.results[r]["y"] for r in range(NCORES)], axis=0)
    return out.astype(np.float32)
